# revision 1
# baseline (speedup 1.0000x reference)
"""AttentiveFP GNN forward pass on 8 Trainium2 NeuronCores (Bass/Tile).

Strategy
--------
Molecules are sharded contiguously across 8 cores (batch is sorted).  Each
core's atoms are laid out in a padded node space where each 256-molecule
block starts at a fixed offset (identical schedule on every core, as required
for a shared SPMD NEFF).  Edges are owned by the core that owns their dst
atom and sorted by dst.  Per-edge work runs in 128-slot sub-tiles grouped by
256-node superwindows; segment softmax + weighted aggregation use
indicator-matrix matmuls accumulating in PSUM, normalized per node
(h[n] = sum_e exp(a_e) m_e / sum_e exp(a_e)).  Layer-1 source features are
host-pre-permuted into slot order (pure input data movement).  Between layer
1 and 2 the updated node features are AllGathered across cores; layer 2
fetches x2[src] rows with one [128,1]-indexed indirect DMA per sub-tile (the
only dynamic gather).  The molecule readout (2 timesteps) is fully local and
gather-free.
"""

import math
import sys

sys.path.insert(0, "/opt/trn_rl_repo")

import numpy as np

import concourse.bass as bass
import concourse.mybir as mybir
import concourse.tile as tile
from concourse import bacc
from concourse.bass_utils import run_bass_kernel_spmd

F32 = mybir.dt.float32
I32 = mybir.dt.int32
AF = mybir.ActivationFunctionType
ALU = mybir.AluOpType

P = 128
SW = 256          # nodes per superwindow (2 psum halves)
MGN = 1024        # nodes per megagroup (GRU batch)
NCORES = 8
H = 128
NUM_TIMESTEPS = 2

FT = F32          # feature dtype for tables / matmul operands


class Cfg:
    pass


def _round_up(x, m):
    return (x + m - 1) // m * m


def _pack_slots(arr):
    """[S] -> [P, S//P] with slot s=j*P+p stored at [p, j]."""
    s = arr.shape[0]
    return np.ascontiguousarray(arr.reshape(s // P, P).T)


def preprocess(inputs, n_cores=NCORES):
    x = np.asarray(inputs["x"], np.float32)
    ea = np.asarray(inputs["edge_attr"], np.float32)
    ei = np.asarray(inputs["edge_index"], np.int32)
    batch = np.asarray(inputs["batch"], np.int32)
    n_atoms, in_dim = x.shape
    ed = ea.shape[1]
    n_mols = int(batch.max()) + 1

    b_core = max(1, n_mols // n_cores)
    mol_bounds = np.searchsorted(batch, np.arange(0, n_cores + 1) * b_core)
    mol_bounds[-1] = n_atoms
    a0 = mol_bounds[:-1].astype(np.int64)

    b_pad = _round_up(b_core + 1, SW)
    n_msw = b_pad // SW

    # block = 256 consecutive molecules; block row-start is FIXED across cores
    blk_cnt = np.zeros((n_cores, n_msw), np.int64)
    for c in range(n_cores):
        bl = batch[mol_bounds[c]:mol_bounds[c + 1]] - c * b_core
        blk_cnt[c] = np.bincount(bl // SW, minlength=n_msw)
    s_blk = _round_up(int(blk_cnt.max()), MGN)
    n_pad = n_msw * s_blk
    w_n = n_pad // P
    n_sw = n_pad // SW

    cfg = Cfg()
    cfg.n_atoms, cfg.in_dim, cfg.ed = n_atoms, in_dim, ed
    cfg.in_pad = in_dim + 1
    cfg.b_core, cfg.b_pad, cfg.n_msw, cfg.s_blk = b_core, b_pad, n_msw, s_blk
    cfg.n_pad, cfg.w_n, cfg.n_sw = n_pad, w_n, n_sw
    cfg.n_mg = n_pad // MGN
    cfg.mw_n = b_pad // P
    cfg.nch = 8
    cfg.ch_sub = 32
    rch = n_pad // cfg.nch

    # padded node position of each atom
    atom_owner = np.clip(
        np.searchsorted(mol_bounds, np.arange(n_atoms), side="right") - 1,
        0, n_cores - 1)
    bl_all = batch - atom_owner * b_core
    msw_all = bl_all // SW
    blk_start = np.zeros((n_cores, n_msw), np.int64)
    blk_start[:, 1:] = np.cumsum(blk_cnt, axis=1)[:, :-1]
    core_a0 = a0[atom_owner]
    within = np.arange(n_atoms) - core_a0 - blk_start[atom_owner, msw_all]
    node_pos = msw_all * s_blk + within      # padded position within core

    src, dst = ei[0].astype(np.int64), ei[1].astype(np.int64)
    e_owner = np.clip(np.searchsorted(mol_bounds, dst, side="right") - 1,
                      0, n_cores - 1)
    dst_pos = node_pos[dst]

    # agf row of a global atom (allgather chunk layout)
    kk = node_pos // rch
    agf_row = kk * (n_cores * rch) + atom_owner * rch + (node_pos - kk * rch)

    counts = np.zeros((n_cores, n_sw), np.int64)
    per_core = []
    for c in range(n_cores):
        sel = np.nonzero(e_owner == c)[0]
        dp = dst_pos[sel]
        order = np.argsort(dp, kind="stable")
        sel = sel[order]
        dp = dp[order]
        counts[c] = np.bincount(dp // SW, minlength=n_sw)
        per_core.append((sel, dp))
    esub_sw = np.maximum(1, np.ceil(counts.max(axis=0) / P).astype(np.int64))
    s_e = int(esub_sw.sum()) * P
    cfg.esub_sw = esub_sw
    cfg.s_e = s_e

    ftnp = np.dtype(mybir.dt.np(FT))
    xraw_pad = np.zeros((n_atoms, cfg.in_pad), np.float32)
    xraw_pad[:, :in_dim] = x

    in_maps = []
    for c in range(n_cores):
        sel, dp = per_core[c]
        slot_src = np.zeros(s_e, np.int64)
        slot_agf = np.zeros(s_e, np.int32)
        slot_dstrel = np.full(s_e, -1.0, np.float32)
        slot_ea = np.zeros((s_e, ed), np.float32)
        estart = np.concatenate([[0], np.cumsum(counts[c])]).astype(np.int64)
        base = 0
        for sw in range(n_sw):
            cnt = int(counts[c, sw])
            lo, hi = estart[sw], estart[sw] + cnt
            slot_src[base:base + cnt] = src[sel[lo:hi]]
            slot_agf[base:base + cnt] = agf_row[src[sel[lo:hi]]]
            slot_dstrel[base:base + cnt] = dp[lo:hi] - sw * SW
            slot_ea[base:base + cnt] = ea[sel[lo:hi]]
            base += int(esub_sw[sw]) * P
        assert base == s_e

        xrt = np.zeros((cfg.in_pad, n_pad), np.float32)
        amask = atom_owner == c
        xrt[:in_dim, node_pos[amask]] = x[amask].T

        molrel = np.full(n_pad, -1.0, np.float32)
        molrel[node_pos[amask]] = (bl_all[amask] - msw_all[amask] * SW)

        in_maps.append({
            "xgT": np.ascontiguousarray(xraw_pad[slot_src].T).astype(ftnp),
            "xrawT_own": xrt,
            "eaT": np.ascontiguousarray(slot_ea.T).astype(ftnp),
            "gidx2": _pack_slots(slot_agf),
            "dstrel": _pack_slots(slot_dstrel),
            "dstrel_row": slot_dstrel.reshape(1, -1).copy(),
            "molrel": _pack_slots(molrel),
            "molrel_row": molrel.reshape(1, -1).copy(),
        })

    # ---- weights / consts ----
    g = lambda q: np.asarray(inputs[q], np.float32)
    wm = {}
    wlin1t = np.zeros((cfg.in_pad, H), np.float32)
    wlin1t[:in_dim] = g("W_lin1").T
    wm["Wlin1T"] = wlin1t.astype(ftnp)
    wm["W1aT"] = np.ascontiguousarray(g("gate_W1")[:, :H].T).astype(ftnp)
    wm["W1bT"] = np.ascontiguousarray(g("gate_W1")[:, H:H + ed].T).astype(ftnp)
    wm["W2T"] = np.ascontiguousarray(g("gate_W2").T).astype(ftnp)
    wm["Wih1T"] = np.ascontiguousarray(g("gru1_Wih").T).astype(ftnp)
    wm["Whh1T"] = np.ascontiguousarray(g("gru1_Whh").T).astype(ftnp)
    wm["convWT"] = np.ascontiguousarray(g("conv_W").T).astype(ftnp)
    wm["Wih2T"] = np.ascontiguousarray(g("gru2_Wih").T).astype(ftnp)
    wm["Whh2T"] = np.ascontiguousarray(g("gru2_Whh").T).astype(ftnp)
    wm["molWT"] = np.ascontiguousarray(g("mol_W").T).astype(ftnp)
    wm["WihmT"] = np.ascontiguousarray(g("grum_Wih").T).astype(ftnp)
    wm["WhhmT"] = np.ascontiguousarray(g("grum_Whh").T).astype(ftnp)
    wm["Wlin2T"] = np.ascontiguousarray(g("W_lin2").T).astype(ftnp)
    wm["WheadT"] = np.ascontiguousarray(g("W_head").T).astype(ftnp)

    cols = {}

    def col(name, v):
        cols[name] = np.asarray(v, np.float32).reshape(H)

    col("b1", g("b_lin1"))
    col("attl", g("gate_att_l"))
    col("attr", g("gate_att_r"))
    col("gbias", g("gate_bias"))
    col("cattsrc", g("conv_W").T @ g("conv_att_src"))
    col("cattdst", g("conv_W").T @ g("conv_att_dst"))
    col("cbias", g("conv_bias"))
    col("cattmsrc", g("mol_W").T @ g("mol_att_src"))
    col("cattmdst", g("mol_W").T @ g("mol_att_dst"))
    col("molbias", g("mol_bias"))
    col("b2", g("b_lin2"))
    col("iop_lo", np.arange(P, dtype=np.float32))
    col("iop_hi", np.arange(P, dtype=np.float32) + P)
    for tag, pre in (("1", "gru1"), ("2", "gru2"), ("m", "grum")):
        bih = g(pre + "_bih")
        bhh = g(pre + "_bhh")
        col("brz_r" + tag, bih[:H] + bhh[:H])
        col("brz_z" + tag, bih[H:2 * H] + bhh[H:2 * H])
        col("bihn" + tag, bih[2 * H:])
        col("bhhn" + tag, bhh[2 * H:])
    order = sorted(cols)
    wm["cvec"] = np.stack([cols[q] for q in order], axis=1)
    cvec_idx = {q: i for i, q in enumerate(order)}

    iota = np.arange(P, dtype=np.float32)
    wm["iota_lo"] = np.tile(iota[None, :], (P, 1))
    wm["iota_hi"] = wm["iota_lo"] + P
    wm["identf32"] = np.eye(P, dtype=np.float32)
    wm["identity"] = np.eye(P, dtype=ftnp)

    for m in in_maps:
        m.update(wm)

    meta = {"cvec_idx": cvec_idx,
            "b_head": float(np.asarray(inputs["b_head"]).reshape(-1)[0])}
    return cfg, in_maps, meta


# ---------------------------------------------------------------------------

class Builder:
    def __init__(self, cfg, cvec_idx, b_head):
        self.cfg = cfg
        self.cvec_idx = cvec_idx
        self.b_head = b_head
        self.nc = bacc.Bacc("TRN2", target_bir_lowering=False, debug=False,
                            num_devices=NCORES)

    def cc(self, name):
        i = self.cvec_idx[name]
        return self.scvec[:, i:i + 1]

    def declare(self):
        nc, cfg = self.nc, self.cfg
        ei = lambda nm, sh, dt: nc.dram_tensor(nm, sh, dt, kind="ExternalInput")
        self.xgT = ei("xgT", [cfg.in_pad, cfg.s_e], FT)
        self.xrawT_own = ei("xrawT_own", [cfg.in_pad, cfg.n_pad], F32)
        self.eaT = ei("eaT", [cfg.ed, cfg.s_e], FT)
        self.gidx2 = ei("gidx2", [P, cfg.s_e // P], I32)
        self.dstrel = ei("dstrel", [P, cfg.s_e // P], F32)
        self.dstrel_row = ei("dstrel_row", [1, cfg.s_e], F32)
        self.molrel = ei("molrel", [P, cfg.n_pad // P], F32)
        self.molrel_row = ei("molrel_row", [1, cfg.n_pad], F32)
        wn = {}
        for nm, sh in (("Wlin1T", [cfg.in_pad, H]), ("W1aT", [H, H]),
                       ("W1bT", [cfg.ed, H]), ("W2T", [H, H]),
                       ("Wih1T", [H, 3 * H]), ("Whh1T", [H, 3 * H]),
                       ("convWT", [H, H]), ("Wih2T", [H, 3 * H]),
                       ("Whh2T", [H, 3 * H]), ("molWT", [H, H]),
                       ("WihmT", [H, 3 * H]), ("WhhmT", [H, 3 * H]),
                       ("Wlin2T", [H, H]), ("WheadT", [H, 1]),
                       ("identity", [P, P])):
            wn[nm] = ei(nm, sh, FT)
        for nm, sh in (("cvec", [P, len(self.cvec_idx)]),
                       ("identf32", [P, P]), ("iota_lo", [P, P]),
                       ("iota_hi", [P, P])):
            wn[nm] = ei(nm, sh, F32)
        self.win = wn
        self.outp = nc.dram_tensor("out", [1, cfg.b_pad], F32,
                                   kind="ExternalOutput")
        self.x1T_d = nc.dram_tensor("x1T_d", [P, cfg.n_pad], F32)
        self.x2aug_d = nc.dram_tensor("x2aug_d", [cfg.n_pad, H + 1], FT)
        self.x2T_d = nc.dram_tensor("x2T_d", [P, cfg.n_pad], F32)
        self.agf_d = nc.dram_tensor("agf_d", [NCORES * cfg.n_pad, H + 1], FT,
                                    addr_space="Shared")
        self.x3aug_d = nc.dram_tensor("x3aug_d", [cfg.n_pad, H + 1], FT)

    def load_weights(self, tc, stack):
        nc = self.nc
        self.pw = stack.enter_context(tc.tile_pool(name="weights", bufs=1))
        self.pin = stack.enter_context(tc.tile_pool(name="pinned", bufs=1))

        def lc(nm):
            h = self.win[nm]
            t = self.pw.tile(list(h.shape), h.dtype, tag=nm, name=nm)
            nc.sync.dma_start(out=t[:, :], in_=h[:, :])
            return t

        self.sW = {nm: lc(nm) for nm in self.win}
        self.scvec = self.sW["cvec"]
        self.r1sb = self.pin.tile([P, self.cfg.w_n], F32, tag="r1sb",
                                  name="r1sb")
        self.a2sb = self.pin.tile([P, self.cfg.w_n], F32, tag="a2sb",
                                  name="a2sb")

    # ---------------- phase 0: lin1 + r1 on own atoms ----------------
    def phase0(self, tc):
        nc, cfg = self.nc, self.cfg
        WPM = MGN // P
        with tc.tile_pool(name="p0", bufs=2) as po, \
             tc.tile_pool(name="p0ps", bufs=2, space="PSUM") as pps:
            for mg in range(cfg.n_mg):
                m0 = mg * MGN
                xrt = po.tile([cfg.in_pad, MGN], F32, tag="xrt", name="xrt")
                nc.sync.dma_start(out=xrt[:, :],
                                  in_=self.xrawT_own[:, m0:m0 + MGN])
                x1mg = po.tile([P, MGN], F32, tag="x1mg", name="x1mg")
                for w8 in range(WPM):
                    ps = pps.tile([P, P], F32, tag="p0ps", name="p0ps",
                                  space="PSUM")
                    nc.tensor.matmul(ps[:, :], lhsT=self.sW["Wlin1T"][:, :],
                                     rhs=xrt[:, w8 * P:(w8 + 1) * P],
                                     start=True, stop=True)
                    nc.scalar.activation(x1mg[:, w8 * P:(w8 + 1) * P], ps[:, :],
                                         AF.Lrelu, bias=self.cc("b1"),
                                         alpha=0.01)
                    psr = pps.tile([P, P], F32, tag="p0ps", name="psr",
                                   space="PSUM")
                    nc.tensor.matmul(psr[:, 0:1],
                                     lhsT=x1mg[:, w8 * P:(w8 + 1) * P],
                                     rhs=self.cc("attr"), start=True, stop=True)
                    nc.vector.tensor_copy(
                        self.r1sb[:, mg * WPM + w8:mg * WPM + w8 + 1],
                        psr[:, 0:1])
                nc.sync.dma_start(out=self.x1T_d[:, m0:m0 + MGN], in_=x1mg[:, :])

    # ---------------- GRU (feature-major, psum-accumulated) ----------------
    def gru(self, pool_sb, pool_ps, WihT, WhhT, tg, hT_ap, xprevT_ap, outT_ap,
            width):
        nc = self.nc
        nq = math.ceil(width / 512)
        for q in range(nq):
            sl = slice(q * 512, min((q + 1) * 512, width))
            qn = sl.stop - sl.start
            prz = pool_ps.tile([P, 512], F32, tag="gps", name="prz",
                               space="PSUM")
            nc.tensor.matmul(prz[:, :qn], lhsT=WihT[:, 0:H], rhs=hT_ap[:, sl],
                             start=True, stop=False)
            nc.tensor.matmul(prz[:, :qn], lhsT=WhhT[:, 0:H],
                             rhs=xprevT_ap[:, sl], start=False, stop=True)
            r = pool_sb.tile([P, 512], F32, tag="g_r", name="g_r")
            nc.scalar.activation(r[:, :qn], prz[:, :qn], AF.Sigmoid,
                                 bias=self.cc("brz_r" + tg))
            pz = pool_ps.tile([P, 512], F32, tag="gps", name="pz", space="PSUM")
            nc.tensor.matmul(pz[:, :qn], lhsT=WihT[:, H:2 * H], rhs=hT_ap[:, sl],
                             start=True, stop=False)
            nc.tensor.matmul(pz[:, :qn], lhsT=WhhT[:, H:2 * H],
                             rhs=xprevT_ap[:, sl], start=False, stop=True)
            z = pool_sb.tile([P, 512], F32, tag="g_z", name="g_z")
            nc.scalar.activation(z[:, :qn], pz[:, :qn], AF.Sigmoid,
                                 bias=self.cc("brz_z" + tg))
            pgn = pool_ps.tile([P, 512], F32, tag="gps", name="pgn",
                               space="PSUM")
            nc.tensor.matmul(pgn[:, :qn], lhsT=WihT[:, 2 * H:3 * H],
                             rhs=hT_ap[:, sl], start=True, stop=True)
            pgh = pool_ps.tile([P, 512], F32, tag="gps", name="pgh",
                               space="PSUM")
            nc.tensor.matmul(pgh[:, :qn], lhsT=WhhT[:, 2 * H:3 * H],
                             rhs=xprevT_ap[:, sl], start=True, stop=True)
            hn = pool_sb.tile([P, 512], F32, tag="g_t", name="hn")
            nc.vector.tensor_scalar(out=hn[:, :qn], in0=pgh[:, :qn],
                                    scalar1=self.cc("bhhn" + tg), scalar2=None,
                                    op0=ALU.add)
            rn = pool_sb.tile([P, 512], F32, tag="g_t", name="rn")
            nc.vector.tensor_tensor(out=rn[:, :qn], in0=r[:, :qn],
                                    in1=hn[:, :qn], op=ALU.mult)
            pre_n = pool_sb.tile([P, 512], F32, tag="g_t", name="pre_n")
            nc.vector.tensor_tensor(out=pre_n[:, :qn], in0=pgn[:, :qn],
                                    in1=rn[:, :qn], op=ALU.add)
            n_ = pool_sb.tile([P, 512], F32, tag="g_n", name="g_n")
            nc.scalar.activation(n_[:, :qn], pre_n[:, :qn], AF.Tanh,
                                 bias=self.cc("bihn" + tg))
            d = pool_sb.tile([P, 512], F32, tag="g_t", name="d")
            nc.vector.tensor_tensor(out=d[:, :qn], in0=xprevT_ap[:, sl],
                                    in1=n_[:, :qn], op=ALU.subtract)
            zd = pool_sb.tile([P, 512], F32, tag="g_t", name="zd")
            nc.vector.tensor_tensor(out=zd[:, :qn], in0=z[:, :qn],
                                    in1=d[:, :qn], op=ALU.mult)
            xs = pool_sb.tile([P, 512], F32, tag="g_t", name="xs")
            nc.vector.tensor_tensor(out=xs[:, :qn], in0=n_[:, :qn],
                                    in1=zd[:, :qn], op=ALU.add)
            nc.vector.tensor_scalar(out=outT_ap[:, sl], in0=xs[:, :qn],
                                    scalar1=0.0, scalar2=None, op0=ALU.max)

    def elu(self, pool_sb, ps_ap, bias_col, out_ap, qn):
        nc = self.nc
        hb = pool_sb.tile([P, 512], F32, tag="e_hb", name="e_hb")
        nc.vector.tensor_scalar(out=hb[:, :qn], in0=ps_ap, scalar1=bias_col,
                                scalar2=None, op0=ALU.add)
        el = pool_sb.tile([P, 512], F32, tag="e_t", name="e_el")
        nc.vector.tensor_scalar(out=el[:, :qn], in0=hb[:, :qn], scalar1=0.0,
                                scalar2=None, op0=ALU.min)
        ex = pool_sb.tile([P, 512], F32, tag="e_ex", name="e_ex")
        nc.scalar.activation(ex[:, :qn], el[:, :qn], AF.Exp)
        mx = pool_sb.tile([P, 512], F32, tag="e_t", name="e_mx")
        nc.vector.tensor_scalar(out=mx[:, :qn], in0=hb[:, :qn], scalar1=0.0,
                                scalar2=None, op0=ALU.max)
        sm = pool_sb.tile([P, 512], F32, tag="e_sm", name="e_sm")
        nc.vector.tensor_tensor(out=sm[:, :qn], in0=mx[:, :qn], in1=ex[:, :qn],
                                op=ALU.add)
        nc.vector.tensor_scalar(out=out_ap, in0=sm[:, :qn], scalar1=-1.0,
                                scalar2=None, op0=ALU.add)

    # ---------------- edge layer (1 or 2) ----------------
    def edge_layer(self, tc, layer):
        nc, cfg = self.nc, self.cfg
        WPM = MGN // P
        esub = cfg.esub_sw
        n_sub_total = cfg.s_e // P
        sub_sw = np.repeat(np.arange(cfg.n_sw), esub)
        sw_first = np.concatenate([[0], np.cumsum(esub)])
        ch_sub = cfg.ch_sub
        sidentf32 = self.sW["identf32"]
        MMX = nc.tensor.matmul

        with tc.tile_pool(name=f"l{layer}g", bufs=2) as pg, \
             tc.tile_pool(name=f"l{layer}rg", bufs=6) as prgather, \
             tc.tile_pool(name=f"l{layer}s", bufs=3) as psub, \
             tc.tile_pool(name=f"l{layer}mg", bufs=2) as pmg, \
             tc.tile_pool(name=f"l{layer}eps", bufs=4, space="PSUM") as pps, \
             tc.tile_pool(name=f"l{layer}hsw", bufs=1, space="PSUM") as phsw, \
             tc.tile_pool(name=f"l{layer}gps", bufs=2, space="PSUM") as ppsg:

            aggT_bufs = {}
            mg_done = {}
            hsw_tiles = {}

            def sw_epilogue(sw, tiles):
                mg = (sw * SW) // MGN
                if mg not in aggT_bufs:
                    aggT_bufs[mg] = pmg.tile([P, MGN], F32, tag="aggT",
                                             name="aggT")
                aggT = aggT_bufs[mg]
                for half, hps in enumerate(tiles):
                    w = 2 * sw + half
                    off = (w * P) % MGN
                    srec = psub.tile([P, 1], F32, tag="srec", name="srec")
                    nc.vector.tensor_scalar(out=srec[:, :], in0=hps[:, H:H + 1],
                                            scalar1=1e-16, scalar2=None,
                                            op0=ALU.add)
                    nc.vector.reciprocal(srec[:, :], srec[:, :])
                    aggN = psub.tile([P, H], F32, tag="aggN", name="aggN")
                    nc.vector.tensor_scalar(out=aggN[:, :], in0=hps[:, :H],
                                            scalar1=srec[:, :], scalar2=None,
                                            op0=ALU.mult)
                    pst = pps.tile([P, P], F32, tag="eps", name="aggps",
                                   space="PSUM")
                    nc.tensor.transpose(pst[:, :], aggN[:, :], sidentf32[:, :])
                    nc.scalar.copy(aggT[:, off:off + P], pst[:, :])
                mg_done[mg] = mg_done.get(mg, 0) + 1
                if mg_done[mg] == MGN // SW:
                    mg_epilogue(mg, aggT_bufs.pop(mg))

            def mg_epilogue(mg, aggT):
                m0 = mg * MGN
                hT = pmg.tile([P, MGN], F32, tag="hT", name="hT")
                for q in range(MGN // 512):
                    ps = ppsg.tile([P, 512], F32, tag="gps", name="wps",
                                   space="PSUM")
                    MMX(ps[:, :], lhsT=(self.sW["W2T"] if layer == 1
                                        else self.sW["convWT"])[:, :],
                        rhs=aggT[:, q * 512:(q + 1) * 512], start=True,
                        stop=True)
                    if layer == 1:
                        self.elu(pmg, ps[:, :], self.cc("gbias"),
                                 hT[:, q * 512:(q + 1) * 512], 512)
                    else:
                        nc.scalar.activation(hT[:, q * 512:(q + 1) * 512],
                                             ps[:, :], AF.Relu,
                                             bias=self.cc("cbias"))
                xprevT = pmg.tile([P, MGN], F32, tag="xprevT", name="xprevT")
                nc.sync.dma_start(
                    out=xprevT[:, :],
                    in_=(self.x1T_d if layer == 1
                         else self.x2T_d)[:, m0:m0 + MGN])
                xnewT = pmg.tile([P, MGN], F32, tag="xnewT", name="xnewT")
                tg = "1" if layer == 1 else "2"
                self.gru(pmg, ppsg, self.sW["Wih" + tg + "T"],
                         self.sW["Whh" + tg + "T"], tg, hT[:, :], xprevT[:, :],
                         xnewT[:, :], MGN)
                aug = pmg.tile([P, WPM * (H + 1)], FT, tag="aug", name="aug")
                for w8 in range(WPM):
                    sl = slice(w8 * P, (w8 + 1) * P)
                    pst = ppsg.tile([P, 512], F32, tag="gps", name="tps",
                                    space="PSUM")
                    nc.tensor.transpose(pst[:, :P], xnewT[:, sl],
                                        sidentf32[:, :])
                    nc.scalar.copy(aug[:, w8 * (H + 1):w8 * (H + 1) + H],
                                   pst[:, :P])
                    psc = ppsg.tile([P, 512], F32, tag="gps", name="cps",
                                    space="PSUM")
                    MMX(psc[:, 0:1], lhsT=xnewT[:, sl],
                        rhs=self.cc("cattsrc") if layer == 1
                        else self.cc("cattmsrc"), start=True, stop=True)
                    nc.vector.tensor_copy(
                        aug[:, w8 * (H + 1) + H:w8 * (H + 1) + H + 1],
                        psc[:, 0:1])
                    if layer == 1:
                        psd = ppsg.tile([P, 512], F32, tag="gps", name="dps",
                                        space="PSUM")
                        MMX(psd[:, 0:1], lhsT=xnewT[:, sl],
                            rhs=self.cc("cattdst"), start=True, stop=True)
                        nc.vector.tensor_copy(
                            self.a2sb[:, mg * WPM + w8:mg * WPM + w8 + 1],
                            psd[:, 0:1])
                aug_d = self.x2aug_d if layer == 1 else self.x3aug_d
                dview = aug_d[m0:m0 + MGN, :].rearrange("(w p) f -> p w f", p=P)
                nc.sync.dma_start(
                    out=dview,
                    in_=aug[:, :].rearrange("p (w f) -> p w f", w=WPM))
                if layer == 1:
                    nc.sync.dma_start(out=self.x2T_d[:, m0:m0 + MGN],
                                      in_=xnewT[:, :])

            for ch in range(math.ceil(n_sub_total / ch_sub)):
                st0 = ch * ch_sub
                st1 = min(st0 + ch_sub, n_sub_total)
                k = st1 - st0
                if layer == 1:
                    xgc = pg.tile([cfg.in_pad, ch_sub * P], FT, tag="xgc",
                                  name="xgc")
                    nc.sync.dma_start(out=xgc[:, :k * P],
                                      in_=self.xgT[:, st0 * P:st1 * P])
                    eac = pg.tile([cfg.ed, ch_sub * P], FT, tag="eac",
                                  name="eac")
                    nc.sync.dma_start(out=eac[:, :k * P],
                                      in_=self.eaT[:, st0 * P:st1 * P])
                drc = pg.tile([P, ch_sub], F32, tag="drc", name="drc")
                nc.sync.dma_start(out=drc[:, :k], in_=self.dstrel[:, st0:st1])
                if layer == 2:
                    gix = pg.tile([P, ch_sub], I32, tag="gix", name="gix")
                    nc.sync.dma_start(out=gix[:, :k],
                                      in_=self.gidx2[:, st0:st1])
                drcrep = pg.tile([P, ch_sub * P], F32, tag="drcrep",
                                 name="drcrep")
                nc.sync.dma_start(
                    out=drcrep[:, :k * P],
                    in_=self.dstrel_row[:, st0 * P:st1 * P].to_broadcast(
                        [P, k * P]))

                for st in range(st0, st1):
                    j = st - st0
                    sw = int(sub_sw[st])
                    first = st == sw_first[sw]
                    last = st == sw_first[sw + 1] - 1
                    if first:
                        hsw_tiles[sw] = (
                            phsw.tile([P, H + 1], F32, tag="hswlo",
                                      name="hswlo", space="PSUM"),
                            phsw.tile([P, H + 1], F32, tag="hswhi",
                                      name="hswhi", space="PSUM"))
                    hlo, hhi = hsw_tiles[sw]

                    mtlo = psub.tile([P, P], FT, tag="mtlo", name="mtlo")
                    nc.vector.tensor_tensor(
                        out=mtlo[:, :],
                        in0=self.cc("iop_lo").to_broadcast([P, P]),
                        in1=drcrep[:, j * P:(j + 1) * P], op=ALU.is_equal)
                    mthi = psub.tile([P, P], FT, tag="mthi", name="mthi")
                    nc.vector.tensor_tensor(
                        out=mthi[:, :],
                        in0=self.cc("iop_hi").to_broadcast([P, P]),
                        in1=drcrep[:, j * P:(j + 1) * P], op=ALU.is_equal)

                    concat = psub.tile([P, H + 1], FT, tag="concat",
                                       name="concat")
                    apsum = pps.tile([P, P], F32, tag="eps", name="apsum",
                                     space="PSUM")
                    if layer == 1:
                        psx = pps.tile([P, P], F32, tag="eps", name="psx",
                                       space="PSUM")
                        MMX(psx[:, :], lhsT=self.sW["Wlin1T"][:, :],
                            rhs=xgc[:, j * P:(j + 1) * P], start=True,
                            stop=True)
                        xj1T = psub.tile([P, P], FT, tag="xj1T", name="xj1T")
                        nc.scalar.activation(xj1T[:, :], psx[:, :], AF.Lrelu,
                                             bias=self.cc("b1"), alpha=0.01)
                        psh = pps.tile([P, P], F32, tag="eps", name="psh",
                                       space="PSUM")
                        MMX(psh[:, :], lhsT=self.sW["W1aT"][:, :],
                            rhs=xj1T[:, :], start=True, stop=False)
                        MMX(psh[:, :], lhsT=self.sW["W1bT"][:, :],
                            rhs=eac[:, j * P:(j + 1) * P], start=False,
                            stop=True)
                        heT = psub.tile([P, P], FT, tag="heT", name="heT")
                        nc.scalar.activation(heT[:, :], psh[:, :], AF.Lrelu,
                                             alpha=0.01)
                        MMX(apsum[:, 0:1], lhsT=heT[:, :], rhs=self.cc("attl"),
                            start=True, stop=False)
                        MMX(apsum[:, 0:1], lhsT=mtlo[:, :],
                            rhs=self.r1sb[:, 2 * sw:2 * sw + 1], start=False,
                            stop=False)
                        MMX(apsum[:, 0:1], lhsT=mthi[:, :],
                            rhs=self.r1sb[:, 2 * sw + 1:2 * sw + 2],
                            start=False, stop=True)
                        a1 = psub.tile([P, 1], F32, tag="a1", name="a1")
                        nc.scalar.activation(a1[:, :], apsum[:, 0:1], AF.Lrelu,
                                             alpha=0.01)
                        nc.scalar.activation(concat[:, H:H + 1], a1[:, :],
                                             AF.Exp)
                        pst2 = pps.tile([P, P], FT, tag="eps", name="pst2",
                                        space="PSUM")
                        nc.tensor.transpose(pst2[:, :], xj1T[:, :],
                                            self.sW["identity"][:, :])
                        nc.vector.tensor_scalar(
                            out=concat[:, :H], in0=pst2[:, :],
                            scalar1=concat[:, H:H + 1], scalar2=None,
                            op0=ALU.mult)
                    else:
                        rg = prgather.tile([P, H + 1], FT, tag="rg", name="rg")
                        nc.gpsimd.indirect_dma_start(
                            out=rg[:, :], out_offset=None,
                            in_=self.agf_d[:, :],
                            in_offset=bass.IndirectOffsetOnAxis(
                                ap=gix[:, j:j + 1], axis=0))
                        MMX(apsum[:, 0:1], lhsT=mtlo[:, :],
                            rhs=self.a2sb[:, 2 * sw:2 * sw + 1], start=True,
                            stop=False)
                        MMX(apsum[:, 0:1], lhsT=mthi[:, :],
                            rhs=self.a2sb[:, 2 * sw + 1:2 * sw + 2],
                            start=False, stop=True)
                        apre = psub.tile([P, 1], F32, tag="a1", name="apre")
                        nc.vector.tensor_tensor(out=apre[:, :],
                                                in0=apsum[:, 0:1],
                                                in1=rg[:, H:H + 1], op=ALU.add)
                        a1 = psub.tile([P, 1], F32, tag="a1b", name="a1b")
                        nc.scalar.activation(a1[:, :], apre[:, :], AF.Lrelu,
                                             alpha=0.01)
                        nc.scalar.activation(concat[:, H:H + 1], a1[:, :],
                                             AF.Exp)
                        nc.vector.tensor_scalar(
                            out=concat[:, :H], in0=rg[:, :H],
                            scalar1=concat[:, H:H + 1], scalar2=None,
                            op0=ALU.mult)

                    mlo = psub.tile([P, P], FT, tag="mlo", name="mlo")
                    nc.vector.tensor_tensor(
                        out=mlo[:, :],
                        in0=drc[:, j:j + 1].to_broadcast([P, P]),
                        in1=self.sW["iota_lo"][:, :], op=ALU.is_equal)
                    mhi = psub.tile([P, P], FT, tag="mhi", name="mhi")
                    nc.vector.tensor_tensor(
                        out=mhi[:, :],
                        in0=drc[:, j:j + 1].to_broadcast([P, P]),
                        in1=self.sW["iota_hi"][:, :], op=ALU.is_equal)
                    MMX(hlo[:, :], lhsT=mlo[:, :], rhs=concat[:, :],
                        start=first, stop=last, skip_group_check=True)
                    MMX(hhi[:, :], lhsT=mhi[:, :], rhs=concat[:, :],
                        start=first, stop=last, skip_group_check=True)
                    if last:
                        sw_epilogue(sw, hsw_tiles.pop(sw))

    def allgather(self):
        nc, cfg = self.nc, self.cfg
        rch = cfg.n_pad // cfg.nch
        for q in range(cfg.nch):
            nc.gpsimd.collective_compute(
                "AllGather", ALU.bypass,
                replica_groups=[list(range(NCORES))],
                ins=[self.x2aug_d[q * rch:(q + 1) * rch, :].opt()],
                outs=[self.agf_d[q * NCORES * rch:(q + 1) * NCORES * rch,
                                 :].opt()])

    # ---------------- readout ----------------
    def readout(self, tc):
        nc, cfg = self.nc, self.cfg
        n_sub_total = cfg.n_pad // P
        sub_per_blk = cfg.s_blk // P
        ch_sub = cfg.ch_sub
        sidentf32 = self.sW["identf32"]
        MMX = nc.tensor.matmul

        with tc.tile_pool(name="ro", bufs=2) as pro, \
             tc.tile_pool(name="roS", bufs=3) as prs, \
             tc.tile_pool(name="roT", bufs=1) as proT, \
             tc.tile_pool(name="rog", bufs=3) as prg, \
             tc.tile_pool(name="rops", bufs=3, space="PSUM") as prps, \
             tc.tile_pool(name="rohm", bufs=1, space="PSUM") as phm:

            outT = proT.tile([P, cfg.b_pad], F32, tag="outT", name="outT")
            admol = [proT.tile([P, cfg.mw_n], F32, tag=f"admol{t}",
                               name=f"admol{t}")
                     for t in range(NUM_TIMESTEPS)]

            def mol_sw_epilogue(tstep, msw, tiles):
                for half, hps in enumerate(tiles):
                    mw = 2 * msw + half
                    off = mw * P
                    if tstep < 0:
                        agg = prg.tile([P, H], F32, tag="maggN", name="magg")
                        nc.vector.tensor_copy(agg[:, :], hps[:, :H])
                        pst = prps.tile([P, P], F32, tag="gps", name="mtps",
                                        space="PSUM")
                        nc.tensor.transpose(pst[:, :], agg[:, :],
                                            sidentf32[:, :])
                        nc.scalar.activation(outT[:, off:off + P], pst[:, :],
                                             AF.Relu)
                        continue
                    srec = prg.tile([P, 1], F32, tag="msrec", name="msrec")
                    nc.vector.tensor_scalar(out=srec[:, :], in0=hps[:, H:H + 1],
                                            scalar1=1e-16, scalar2=None,
                                            op0=ALU.add)
                    nc.vector.reciprocal(srec[:, :], srec[:, :])
                    aggN = prg.tile([P, H], F32, tag="maggN", name="maggN")
                    nc.vector.tensor_scalar(out=aggN[:, :], in0=hps[:, :H],
                                            scalar1=srec[:, :], scalar2=None,
                                            op0=ALU.mult)
                    pst = prps.tile([P, P], F32, tag="gps", name="mtps2",
                                    space="PSUM")
                    nc.tensor.transpose(pst[:, :], aggN[:, :], sidentf32[:, :])
                    aggT = prg.tile([P, P], F32, tag="maggT", name="maggT")
                    nc.scalar.copy(aggT[:, :], pst[:, :])
                    psh = prps.tile([P, P], F32, tag="gps", name="mhps",
                                    space="PSUM")
                    MMX(psh[:, :], lhsT=self.sW["molWT"][:, :], rhs=aggT[:, :],
                        start=True, stop=True)
                    hel = prg.tile([P, P], F32, tag="mhel", name="mhel")
                    self.elu(prg, psh[:, :], self.cc("molbias"), hel[:, :P], P)
                    self.gru(prg, prps, self.sW["WihmT"], self.sW["WhhmT"],
                             "m", hel[:, :], outT[:, off:off + P],
                             outT[:, off:off + P], P)

            def mol_pass(tstep):
                hm_tiles = {}
                for ch in range(math.ceil(n_sub_total / ch_sub)):
                    st0 = ch * ch_sub
                    st1 = min(st0 + ch_sub, n_sub_total)
                    k = st1 - st0
                    rg = pro.tile([P, ch_sub * (H + 1)], FT, tag="rg",
                                  name="rrg")
                    nc.sync.dma_start(
                        out=rg[:, :k * (H + 1)].rearrange(
                            "p (j f) -> p j f", j=k),
                        in_=self.x3aug_d[st0 * P:st1 * P, :].rearrange(
                            "(j p) f -> p j f", p=P))
                    mrl = pro.tile([P, ch_sub], F32, tag="mrl", name="mrl")
                    nc.sync.dma_start(out=mrl[:, :k],
                                      in_=self.molrel[:, st0:st1])
                    if tstep >= 0:
                        mrlrep = pro.tile([P, ch_sub * P], F32, tag="mrlrep",
                                          name="mrlrep")
                        nc.sync.dma_start(
                            out=mrlrep[:, :k * P],
                            in_=self.molrel_row[:, st0 * P:st1 * P]
                            .to_broadcast([P, k * P]))
                    for st in range(st0, st1):
                        j = st - st0
                        msw = st // sub_per_blk
                        first = st % sub_per_blk == 0
                        last = (st + 1) % sub_per_blk == 0
                        if first:
                            hm_tiles[msw] = (
                                phm.tile([P, H + 1], F32, tag="hmlo",
                                         name="hmlo", space="PSUM"),
                                phm.tile([P, H + 1], F32, tag="hmhi",
                                         name="hmhi", space="PSUM"))
                        hlo, hhi = hm_tiles[msw]
                        if tstep >= 0:
                            mtlo = prs.tile([P, P], FT, tag="mmtlo",
                                            name="mmtlo")
                            nc.vector.tensor_tensor(
                                out=mtlo[:, :],
                                in0=self.cc("iop_lo").to_broadcast([P, P]),
                                in1=mrlrep[:, j * P:(j + 1) * P],
                                op=ALU.is_equal)
                            mthi = prs.tile([P, P], FT, tag="mmthi",
                                            name="mmthi")
                            nc.vector.tensor_tensor(
                                out=mthi[:, :],
                                in0=self.cc("iop_hi").to_broadcast([P, P]),
                                in1=mrlrep[:, j * P:(j + 1) * P],
                                op=ALU.is_equal)
                            apsum = prps.tile([P, P], F32, tag="gps",
                                              name="mapsum", space="PSUM")
                            MMX(apsum[:, 0:1], lhsT=mtlo[:, :],
                                rhs=admol[tstep][:, 2 * msw:2 * msw + 1],
                                start=True, stop=False)
                            MMX(apsum[:, 0:1], lhsT=mthi[:, :],
                                rhs=admol[tstep][:, 2 * msw + 1:2 * msw + 2],
                                start=False, stop=True)
                            apre = prs.tile([P, 1], F32, tag="ma1",
                                            name="mapre")
                            nc.vector.tensor_tensor(
                                out=apre[:, :], in0=apsum[:, 0:1],
                                in1=rg[:, j * (H + 1) + H:j * (H + 1) + H + 1],
                                op=ALU.add)
                            a1 = prs.tile([P, 1], F32, tag="ma1b", name="ma1b")
                            nc.scalar.activation(a1[:, :], apre[:, :], AF.Lrelu,
                                                 alpha=0.01)
                            concat = prs.tile([P, H + 1], FT, tag="mconcat",
                                              name="mconcat")
                            nc.scalar.activation(concat[:, H:H + 1], a1[:, :],
                                                 AF.Exp)
                            nc.vector.tensor_scalar(
                                out=concat[:, :H],
                                in0=rg[:, j * (H + 1):j * (H + 1) + H],
                                scalar1=concat[:, H:H + 1], scalar2=None,
                                op0=ALU.mult)
                            rhs = concat[:, :]
                        else:
                            rhs = rg[:, j * (H + 1):(j + 1) * (H + 1)]
                        mlo = prs.tile([P, P], FT, tag="mmlo", name="mmlo")
                        nc.vector.tensor_tensor(
                            out=mlo[:, :],
                            in0=mrl[:, j:j + 1].to_broadcast([P, P]),
                            in1=self.sW["iota_lo"][:, :], op=ALU.is_equal)
                        mhi = prs.tile([P, P], FT, tag="mmhi", name="mmhi")
                        nc.vector.tensor_tensor(
                            out=mhi[:, :],
                            in0=mrl[:, j:j + 1].to_broadcast([P, P]),
                            in1=self.sW["iota_hi"][:, :], op=ALU.is_equal)
                        MMX(hlo[:, :], lhsT=mlo[:, :], rhs=rhs, start=first,
                            stop=last, skip_group_check=True)
                        MMX(hhi[:, :], lhsT=mhi[:, :], rhs=rhs, start=first,
                            stop=last, skip_group_check=True)
                        if last:
                            mol_sw_epilogue(tstep, msw, hm_tiles.pop(msw))

            def write_admol(tstep):
                for mw in range(cfg.mw_n):
                    ps = prps.tile([P, P], F32, tag="gps", name="amps",
                                   space="PSUM")
                    MMX(ps[:, 0:1], lhsT=outT[:, mw * P:(mw + 1) * P],
                        rhs=self.cc("cattmdst"), start=True, stop=True)
                    nc.vector.tensor_copy(admol[tstep][:, mw:mw + 1],
                                          ps[:, 0:1])

            import os
            ro_lvl = int(os.environ.get("K_RO", "3"))
            mol_pass(-1)
            if ro_lvl >= 2:
                for t in range(min(NUM_TIMESTEPS, ro_lvl - 1)):
                    write_admol(t)
                    mol_pass(t)

            predsb = proT.tile([1, cfg.b_pad], F32, tag="pred", name="pred")
            for q in range(math.ceil(cfg.b_pad / 512)):
                qs = slice(q * 512, min((q + 1) * 512, cfg.b_pad))
                qn = qs.stop - qs.start
                ps = prps.tile([P, 512], F32, tag="gps", name="finps",
                               space="PSUM")
                MMX(ps[:, :qn], lhsT=self.sW["Wlin2T"][:, :], rhs=outT[:, qs],
                    start=True, stop=True)
                o2 = pro.tile([P, 512], F32, tag="o2", name="o2")
                nc.vector.tensor_scalar(out=o2[:, :qn], in0=ps[:, :qn],
                                        scalar1=self.cc("b2"), scalar2=None,
                                        op0=ALU.add)
                ph = prps.tile([P, 512], F32, tag="gps", name="finph",
                               space="PSUM")
                MMX(ph[0:1, :qn], lhsT=self.sW["WheadT"][:, :], rhs=o2[:, :qn],
                    start=True, stop=True)
                nc.vector.tensor_scalar(out=predsb[:, qs], in0=ph[0:1, :qn],
                                        scalar1=float(self.b_head),
                                        scalar2=None, op0=ALU.add)
            nc.sync.dma_start(out=self.outp[:, :], in_=predsb[:, :])

    def build(self, phases=5):
        from contextlib import ExitStack
        self.declare()
        with tile.TileContext(self.nc) as tc:
            with ExitStack() as stack:
                self.load_weights(tc, stack)
                self.phase0(tc)
                if phases >= 2:
                    self.edge_layer(tc, 1)
                if phases >= 3:
                    self.allgather()
                if phases >= 4:
                    self.edge_layer(tc, 2)
                if phases >= 5:
                    self.readout(tc)
                else:
                    with tc.tile_pool(name="stub", bufs=1) as pstub:
                        z = pstub.tile([1, self.cfg.b_pad], F32, tag="z",
                                       name="z")
                        self.nc.gpsimd.memset(z[:, :], 0.0)
                        self.nc.sync.dma_start(out=self.outp[:, :],
                                               in_=z[:, :])
        self.nc.finalize()
        return self.nc


def build(cfg, cvec_idx, b_head):
    import os
    return Builder(cfg, cvec_idx, b_head).build(
        phases=int(os.environ.get("K_PHASES", "5")))


def kernel(**inputs):
    cfg, in_maps, meta = preprocess(inputs, NCORES)
    nc = build(cfg, meta["cvec_idx"], meta["b_head"])
    res = run_bass_kernel_spmd(nc, in_maps, core_ids=list(range(NCORES)))
    outs = []
    for c in range(NCORES):
        outs.append(res.results[c]["out"].reshape(-1)[:cfg.b_core])
    return np.concatenate(outs).astype(np.float32)



# revision 45
# speedup vs baseline: 3.3316x; 3.3316x over previous
"""AttentiveFP GNN forward pass on 8 Trainium2 NeuronCores (Bass/Tile).

Strategy (v2 — bf16 datapath)
-----------------------------
Molecules sharded contiguously across 8 cores (batch is sorted).  Atoms live
in a padded node space (256-molecule blocks at fixed offsets, identical
schedule on every core for the shared SPMD NEFF).  Edges owned by dst core,
sorted by dst, processed in 128-slot subtiles grouped by 128-node windows.
Segment softmax + weighted aggregation via indicator-matrix matmuls in PSUM.

v2 changes vs baseline:
 - bf16 operands everywhere on the matmul/DVE path (PE 4x, DVE 2-4x, half DMA)
 - single activation-table set: sigmoid done as 0.5*tanh(0.5x)+0.5
 - wide N=512 matmuls for lin1/gate/GRU/W2; batched per-chunk alpha lrelu/exp
 - SW=128 node windows: one mask + one scatter matmul per subtile
 - mask ops via tensor_scalar is_equal; scatter masks on the Pool engine
 - column reductions (r1/a2/attsrc) batched per-mg into [128,8] PSUM tiles
 - AllGather payload in bf16
"""

import math
import sys

sys.path.insert(0, "/opt/trn_rl_repo")

import numpy as np

import concourse.bass as bass
import concourse.mybir as mybir
import concourse.tile as tile
from concourse import bacc
from concourse.bass_utils import run_bass_kernel_spmd

F32 = mybir.dt.float32
BF = mybir.dt.bfloat16
I32 = mybir.dt.int32
AF = mybir.ActivationFunctionType
ALU = mybir.AluOpType

P = 128
SW = 128          # nodes per superwindow (1 psum tile)
MBLK = 256        # molecules per block (readout scatter granularity)
MGN = 1024        # nodes per megagroup (GRU batch)
NCORES = 8
H = 128
NUM_TIMESTEPS = 2


class Cfg:
    pass


def _round_up(x, m):
    return (x + m - 1) // m * m


def _pack_slots(arr):
    """[S] -> [P, S//P] with slot s=j*P+p stored at [p, j]."""
    s = arr.shape[0]
    return np.ascontiguousarray(arr.reshape(s // P, P).T)


def preprocess(inputs, n_cores=NCORES):
    bfnp = np.dtype(mybir.dt.np(BF))
    x = np.asarray(inputs["x"], np.float32)
    ea = np.asarray(inputs["edge_attr"], np.float32)
    ei = np.asarray(inputs["edge_index"], np.int32)
    batch = np.asarray(inputs["batch"], np.int32)
    n_atoms, in_dim = x.shape
    ed = ea.shape[1]
    n_mols = int(batch.max()) + 1

    b_core = max(1, n_mols // n_cores)
    mol_bounds = np.searchsorted(batch, np.arange(0, n_cores + 1) * b_core)
    mol_bounds[-1] = n_atoms
    a0 = mol_bounds[:-1].astype(np.int64)

    b_pad = _round_up(b_core + 1, MBLK)
    n_msw = b_pad // MBLK

    # block = 256 consecutive molecules; block row-start is FIXED across cores
    blk_cnt = np.zeros((n_cores, n_msw), np.int64)
    for c in range(n_cores):
        bl = batch[mol_bounds[c]:mol_bounds[c + 1]] - c * b_core
        blk_cnt[c] = np.bincount(bl // MBLK, minlength=n_msw)
    s_blk = _round_up(int(blk_cnt.max()), MGN)
    n_pad = n_msw * s_blk
    w_n = n_pad // P
    n_sw = n_pad // SW          # == w_n since SW == P

    cfg = Cfg()
    cfg.n_atoms, cfg.in_dim, cfg.ed = n_atoms, in_dim, ed
    cfg.in_pad = in_dim + 1
    cfg.b_core, cfg.b_pad, cfg.n_msw, cfg.s_blk = b_core, b_pad, n_msw, s_blk
    cfg.n_pad, cfg.w_n, cfg.n_sw = n_pad, w_n, n_sw
    cfg.n_mg = n_pad // MGN
    cfg.mw_n = b_pad // P
    cfg.nch = 8
    cfg.ch_sub = 32
    rch = n_pad // cfg.nch

    # padded node position of each atom
    atom_owner = np.clip(
        np.searchsorted(mol_bounds, np.arange(n_atoms), side="right") - 1,
        0, n_cores - 1)
    bl_all = batch - atom_owner * b_core
    msw_all = bl_all // MBLK
    blk_start = np.zeros((n_cores, n_msw), np.int64)
    blk_start[:, 1:] = np.cumsum(blk_cnt, axis=1)[:, :-1]
    core_a0 = a0[atom_owner]
    within = np.arange(n_atoms) - core_a0 - blk_start[atom_owner, msw_all]
    node_pos = msw_all * s_blk + within      # padded position within core

    src, dst = ei[0].astype(np.int64), ei[1].astype(np.int64)
    e_owner = np.clip(np.searchsorted(mol_bounds, dst, side="right") - 1,
                      0, n_cores - 1)
    dst_pos = node_pos[dst]

    # agf row of a global atom (allgather chunk layout)
    kk = node_pos // rch
    agf_row = kk * (n_cores * rch) + atom_owner * rch + (node_pos - kk * rch)

    counts = np.zeros((n_cores, n_sw), np.int64)
    per_core = []
    for c in range(n_cores):
        sel = np.nonzero(e_owner == c)[0]
        dp = dst_pos[sel]
        order = np.argsort(dp, kind="stable")
        sel = sel[order]
        dp = dp[order]
        counts[c] = np.bincount(dp // SW, minlength=n_sw)
        per_core.append((sel, dp))
    esub_sw = np.maximum(1, np.ceil(counts.max(axis=0) / P).astype(np.int64))
    s_e = int(esub_sw.sum()) * P
    cfg.esub_sw = esub_sw
    cfg.s_e = s_e

    # pad feature column is constant 1.0 so a bias row in Wlin1R folds b_lin1
    # into the edge-major lin1 matmul
    xraw_pad = np.zeros((n_atoms, cfg.in_pad), np.float32)
    xraw_pad[:, :in_dim] = x
    xraw_pad[:, in_dim] = 1.0

    in_maps = []
    for c in range(n_cores):
        sel, dp = per_core[c]
        slot_src = np.zeros(s_e, np.int64)
        slot_agf = np.zeros(s_e, np.int32)
        slot_dstrel = np.full(s_e, -1.0, np.float32)
        slot_ea = np.zeros((s_e, ed), np.float32)
        estart = np.concatenate([[0], np.cumsum(counts[c])]).astype(np.int64)
        base = 0
        for sw in range(n_sw):
            cnt = int(counts[c, sw])
            lo, hi = estart[sw], estart[sw] + cnt
            slot_src[base:base + cnt] = src[sel[lo:hi]]
            slot_agf[base:base + cnt] = agf_row[src[sel[lo:hi]]]
            slot_dstrel[base:base + cnt] = dp[lo:hi] - sw * SW
            slot_ea[base:base + cnt] = ea[sel[lo:hi]]
            base += int(esub_sw[sw]) * P
        assert base == s_e

        xrt = np.zeros((cfg.in_pad, n_pad), np.float32)
        amask = atom_owner == c
        xrt[:in_dim, node_pos[amask]] = x[amask].T

        molrel = np.full(n_pad, -1.0, np.float32)
        molrel[node_pos[amask]] = (bl_all[amask] - msw_all[amask] * MBLK)

        in_maps.append({
            "xgT": np.ascontiguousarray(xraw_pad[slot_src].T).astype(bfnp),
            "xrawT_own": xrt.astype(bfnp),
            "eaT": np.ascontiguousarray(slot_ea.T).astype(bfnp),
            "gidx2": _pack_slots(slot_agf),
            "dstrel": _pack_slots(slot_dstrel),
            "dstrel_row": slot_dstrel.reshape(1, -1).astype(bfnp),
            "molrel": _pack_slots(molrel),
            "molrel_row": molrel.reshape(1, -1).astype(bfnp),
        })

    # ---- weights / consts ----
    g = lambda q: np.asarray(inputs[q], np.float32)
    wm = {}
    wlin1t = np.zeros((cfg.in_pad, H), np.float32)
    wlin1t[:in_dim] = g("W_lin1").T
    wm["Wlin1T"] = wlin1t.astype(bfnp)
    wlin1r = wlin1t.copy()
    wlin1r[in_dim] = g("b_lin1")
    wm["Wlin1R"] = wlin1r.astype(bfnp)
    wm["W1aT"] = np.ascontiguousarray(g("gate_W1")[:, :H].T).astype(bfnp)
    wm["W1bT"] = np.ascontiguousarray(g("gate_W1")[:, H:H + ed].T).astype(bfnp)
    wm["W2T"] = np.ascontiguousarray(g("gate_W2").T).astype(bfnp)
    wm["Wih1T"] = np.ascontiguousarray(g("gru1_Wih").T).astype(bfnp)
    wm["Whh1T"] = np.ascontiguousarray(g("gru1_Whh").T).astype(bfnp)
    wm["convWT"] = np.ascontiguousarray(g("conv_W").T).astype(bfnp)
    wm["Wih2T"] = np.ascontiguousarray(g("gru2_Wih").T).astype(bfnp)
    wm["Whh2T"] = np.ascontiguousarray(g("gru2_Whh").T).astype(bfnp)
    wm["molWT"] = np.ascontiguousarray(g("mol_W").T).astype(bfnp)
    wm["WihmT"] = np.ascontiguousarray(g("grum_Wih").T).astype(bfnp)
    wm["WhhmT"] = np.ascontiguousarray(g("grum_Whh").T).astype(bfnp)
    wm["Wlin2T"] = np.ascontiguousarray(g("W_lin2").T).astype(bfnp)
    wm["WheadT"] = np.ascontiguousarray(g("W_head").T).astype(bfnp)
    wm["identity"] = np.eye(P, dtype=bfnp)

    # f32 per-partition scalar columns (Act bias / DVE tensor_scalar operands)
    cols = {}

    def col(name, v):
        cols[name] = np.asarray(v, np.float32).reshape(H)

    col("b1", g("b_lin1"))
    col("gbias", g("gate_bias"))
    col("cbias", g("conv_bias"))
    col("molbias", g("mol_bias"))
    col("b2", g("b_lin2"))
    col("iop_lo", np.arange(P, dtype=np.float32))
    col("iop_hi", np.arange(P, dtype=np.float32) + P)
    for tag, pre in (("1", "gru1"), ("2", "gru2"), ("m", "grum")):
        bih = g(pre + "_bih")
        bhh = g(pre + "_bhh")
        col("brz_rh" + tag, 0.5 * (bih[:H] + bhh[:H]))
        col("brz_zh" + tag, 0.5 * (bih[H:2 * H] + bhh[H:2 * H]))
        col("bihn" + tag, bih[2 * H:])
        col("bhhn" + tag, bhh[2 * H:])
    order = sorted(cols)
    wm["cvec"] = np.stack([cols[q] for q in order], axis=1)
    cvec_idx = {q: i for i, q in enumerate(order)}

    # bf16 matmul-operand columns
    c16 = {}
    c16["attl"] = g("gate_att_l")
    c16["attr"] = g("gate_att_r")
    c16["cattsrc"] = g("conv_W").T @ g("conv_att_src")
    c16["cattdst"] = g("conv_W").T @ g("conv_att_dst")
    c16["cattmsrc"] = g("mol_W").T @ g("mol_att_src")
    c16["cattmdst"] = g("mol_W").T @ g("mol_att_dst")
    order16 = sorted(c16)
    wm["c16"] = np.stack([c16[q] for q in order16], axis=1).astype(bfnp)
    c16_idx = {q: i for i, q in enumerate(order16)}

    iota = np.arange(P, dtype=np.float32)
    wm["iotaRowLo"] = np.tile(iota[None, :], (P, 1)).astype(bfnp)
    wm["iotaRowHi"] = (np.tile(iota[None, :], (P, 1)) + P).astype(bfnp)

    for m in in_maps:
        m.update(wm)

    meta = {"cvec_idx": cvec_idx, "c16_idx": c16_idx,
            "b_head": float(np.asarray(inputs["b_head"]).reshape(-1)[0])}
    return cfg, in_maps, meta


# ---------------------------------------------------------------------------

class Builder:
    def __init__(self, cfg, cvec_idx, c16_idx, b_head):
        self.cfg = cfg
        self.cvec_idx = cvec_idx
        self.c16_idx = c16_idx
        self.b_head = b_head
        self.nc = bacc.Bacc("TRN2", target_bir_lowering=False, debug=False,
                            num_devices=NCORES)

    def cc(self, name):
        i = self.cvec_idx[name]
        return self.scvec[:, i:i + 1]

    def ch16(self, name):
        i = self.c16_idx[name]
        return self.sc16[:, i:i + 1]

    def declare(self):
        nc, cfg = self.nc, self.cfg
        ei = lambda nm, sh, dt: nc.dram_tensor(nm, sh, dt, kind="ExternalInput")
        self.xgT = ei("xgT", [cfg.in_pad, cfg.s_e], BF)
        self.xrawT_own = ei("xrawT_own", [cfg.in_pad, cfg.n_pad], BF)
        self.eaT = ei("eaT", [cfg.ed, cfg.s_e], BF)
        self.gidx2 = ei("gidx2", [P, cfg.s_e // P], I32)
        self.dstrel = ei("dstrel", [P, cfg.s_e // P], F32)
        self.dstrel_row = ei("dstrel_row", [1, cfg.s_e], BF)
        self.molrel = ei("molrel", [P, cfg.n_pad // P], F32)
        self.molrel_row = ei("molrel_row", [1, cfg.n_pad], BF)
        wn = {}
        for nm, sh in (("Wlin1T", [cfg.in_pad, H]), ("Wlin1R", [cfg.in_pad, H]),
                       ("W1aT", [H, H]),
                       ("W1bT", [cfg.ed, H]), ("W2T", [H, H]),
                       ("Wih1T", [H, 3 * H]), ("Whh1T", [H, 3 * H]),
                       ("convWT", [H, H]), ("Wih2T", [H, 3 * H]),
                       ("Whh2T", [H, 3 * H]), ("molWT", [H, H]),
                       ("WihmT", [H, 3 * H]), ("WhhmT", [H, 3 * H]),
                       ("Wlin2T", [H, H]), ("WheadT", [H, 1]),
                       ("identity", [P, P]),
                       ("c16", [P, len(self.c16_idx)]),
                       ("iotaRowLo", [P, P]), ("iotaRowHi", [P, P])):
            wn[nm] = ei(nm, sh, BF)
        wn["cvec"] = ei("cvec", [P, len(self.cvec_idx)], F32)
        self.win = wn
        self.outp = nc.dram_tensor("out", [1, cfg.b_pad], F32,
                                   kind="ExternalOutput")
        self.x1T_d = nc.dram_tensor("x1T_d", [P, cfg.n_pad], BF)
        self.x2aug_d = nc.dram_tensor("x2aug_d", [cfg.n_pad, H + 1], BF)
        self.x2T_d = nc.dram_tensor("x2T_d", [P, cfg.n_pad], BF)
        self.agf_d = nc.dram_tensor("agf_d", [NCORES * cfg.n_pad, H + 1], BF,
                                    addr_space="Shared")
        self.x3aug_d = nc.dram_tensor("x3aug_d", [cfg.n_pad, H + 1], BF)
        import os
        self.dbg = os.environ.get("K_DBG", "") == "1"
        if self.dbg:
            self.dbg_x1 = nc.dram_tensor("dbg_x1", [P, cfg.n_pad], BF,
                                         kind="ExternalOutput")
            self.dbg_x2aug = nc.dram_tensor("dbg_x2aug", [cfg.n_pad, H + 1],
                                            BF, kind="ExternalOutput")
            self.dbg_x3aug = nc.dram_tensor("dbg_x3aug", [cfg.n_pad, H + 1],
                                            BF, kind="ExternalOutput")
            self.dbg_r1 = nc.dram_tensor("dbg_r1", [P, cfg.w_n], F32,
                                         kind="ExternalOutput")
            self.dbg_a2 = nc.dram_tensor("dbg_a2", [P, cfg.w_n], F32,
                                         kind="ExternalOutput")
            self.dbg_h = nc.dram_tensor("dbg_h", [P, cfg.n_pad], BF,
                                        kind="ExternalOutput")
            self.dbg_den = nc.dram_tensor("dbg_den", [P, cfg.w_n], F32,
                                          kind="ExternalOutput")

    def load_weights(self, tc, stack):
        nc = self.nc
        self.pw = stack.enter_context(tc.tile_pool(name="weights", bufs=1))
        self.pin = stack.enter_context(tc.tile_pool(name="pinned", bufs=1))

        def lc(nm):
            h = self.win[nm]
            t = self.pw.tile(list(h.shape), h.dtype, tag=nm, name=nm)
            nc.sync.dma_start(out=t[:, :], in_=h[:, :])
            return t

        self.sW = {nm: lc(nm) for nm in self.win}
        self.scvec = self.sW["cvec"]
        self.sc16 = self.sW["c16"]
        self.r1sb = self.pin.tile([P, self.cfg.w_n], BF, tag="r1sb",
                                  name="r1sb")
        self.a2sb = self.pin.tile([P, self.cfg.w_n], BF, tag="a2sb",
                                  name="a2sb")

    # ---------------- phase 0: lin1 + r1 on own atoms ----------------
    def phase0(self, tc):
        nc, cfg = self.nc, self.cfg
        WPM = MGN // P
        with tc.tile_pool(name="p0", bufs=2) as po, \
             tc.tile_pool(name="p0ps", bufs=2, space="PSUM") as pps, \
             tc.tile_pool(name="p0pc", bufs=2, space="PSUM") as ppc:
            for mg in range(cfg.n_mg):
                m0 = mg * MGN
                xrt = po.tile([cfg.in_pad, MGN], BF, tag="xrt", name="xrt")
                nc.sync.dma_start(out=xrt[:, :],
                                  in_=self.xrawT_own[:, m0:m0 + MGN])
                x1mg = po.tile([P, MGN], BF, tag="x1mg", name="x1mg")
                for q in range(MGN // 512):
                    ps = pps.tile([P, 512], F32, tag="p0ps", name="p0ps",
                                  space="PSUM")
                    nc.tensor.matmul(ps[:, :], lhsT=self.sW["Wlin1T"][:, :],
                                     rhs=xrt[:, q * 512:(q + 1) * 512],
                                     start=True, stop=True)
                    nc.scalar.activation(x1mg[:, q * 512:(q + 1) * 512],
                                         ps[:, :], AF.Lrelu, bias=self.cc("b1"),
                                         alpha=0.01)
                psr = ppc.tile([P, WPM], F32, tag="psr", name="psr",
                               space="PSUM")
                for w8 in range(WPM):
                    nc.tensor.matmul(psr[:, w8:w8 + 1],
                                     lhsT=x1mg[:, w8 * P:(w8 + 1) * P],
                                     rhs=self.ch16("attr"), start=True,
                                     stop=True, skip_group_check=True)
                nc.vector.tensor_copy(
                    self.r1sb[:, mg * WPM:(mg + 1) * WPM], psr[:, :])
                nc.sync.dma_start(out=self.x1T_d[:, m0:m0 + MGN],
                                  in_=x1mg[:, :])

    # ---------------- GRU (feature-major, bf16, tanh-form sigmoid) ----------
    def gru(self, pool_sb, pool_ps, WihT, WhhT, tg, hT_ap, xprevT_ap, outT_ap,
            width):
        nc = self.nc
        sident = self.sW["identity"]
        nq = math.ceil(width / 512)
        for q in range(nq):
            sl = slice(q * 512, min((q + 1) * 512, width))
            qn = sl.stop - sl.start
            prz = pool_ps.tile([P, 512], F32, tag="gpsA", name="prz",
                               space="PSUM")
            nc.tensor.matmul(prz[:, :qn], lhsT=WihT[:, 0:H], rhs=hT_ap[:, sl],
                             start=True, stop=False)
            nc.tensor.matmul(prz[:, :qn], lhsT=WhhT[:, 0:H],
                             rhs=xprevT_ap[:, sl], start=False, stop=True)
            pz = pool_ps.tile([P, 512], F32, tag="gpsA", name="pz",
                              space="PSUM")
            nc.tensor.matmul(pz[:, :qn], lhsT=WihT[:, H:2 * H],
                             rhs=hT_ap[:, sl], start=True, stop=False)
            nc.tensor.matmul(pz[:, :qn], lhsT=WhhT[:, H:2 * H],
                             rhs=xprevT_ap[:, sl], start=False, stop=True)
            # sigmoid(a) == 0.5*tanh(0.5*a) + 0.5 (keeps one act-table set)
            t_r = pool_sb.tile([P, 512], BF, tag="g_r", name="g_r")
            nc.scalar.activation(t_r[:, :qn], prz[:, :qn], AF.Tanh,
                                 bias=self.cc("brz_rh" + tg), scale=0.5)
            t_z = pool_sb.tile([P, 512], BF, tag="g_z", name="g_z")
            nc.scalar.activation(t_z[:, :qn], pz[:, :qn], AF.Tanh,
                                 bias=self.cc("brz_zh" + tg), scale=0.5)
            pah = pool_ps.tile([P, 512], F32, tag="gpsA", name="pah",
                               space="PSUM")
            nc.tensor.matmul(pah[:, :qn], lhsT=WhhT[:, 2 * H:3 * H],
                             rhs=xprevT_ap[:, sl], start=True, stop=True)
            pin = pool_ps.tile([P, 512], F32, tag="gpsA", name="pin",
                               space="PSUM")
            nc.tensor.matmul(pin[:, :qn], lhsT=WihT[:, 2 * H:3 * H],
                             rhs=hT_ap[:, sl], start=True, stop=False,
                             skip_group_check=True)
            ahn2 = pool_sb.tile([P, 512], BF, tag="g_ah", name="g_ah")
            nc.vector.tensor_scalar(out=ahn2[:, :qn], in0=pah[:, :qn],
                                    scalar1=self.cc("bhhn" + tg), scalar2=None,
                                    op0=ALU.add)
            r2 = pool_sb.tile([P, 512], BF, tag="g_r2", name="g_r2")
            nc.gpsimd.tensor_scalar(out=r2[:, :qn], in0=t_r[:, :qn],
                                    scalar1=0.5, scalar2=0.5, op0=ALU.mult,
                                    op1=ALU.add)
            rn = pool_sb.tile([P, 512], BF, tag="g_rn", name="g_rn")
            nc.vector.tensor_tensor(out=rn[:, :qn], in0=r2[:, :qn],
                                    in1=ahn2[:, :qn], op=ALU.mult)
            nc.tensor.matmul(pin[:, :qn], lhsT=sident[:, :], rhs=rn[:, :qn],
                             start=False, stop=True, skip_group_check=True)
            n_ = pool_sb.tile([P, 512], BF, tag="g_n", name="g_n")
            nc.scalar.activation(n_[:, :qn], pin[:, :qn], AF.Tanh,
                                 bias=self.cc("bihn" + tg))
            d = pool_sb.tile([P, 512], BF, tag="g_d", name="g_d")
            nc.gpsimd.tensor_tensor(out=d[:, :qn], in0=xprevT_ap[:, sl],
                                    in1=n_[:, :qn], op=ALU.subtract)
            z2 = pool_sb.tile([P, 512], BF, tag="g_z2", name="g_z2")
            nc.gpsimd.tensor_scalar(out=z2[:, :qn], in0=t_z[:, :qn],
                                    scalar1=0.5, scalar2=0.5, op0=ALU.mult,
                                    op1=ALU.add)
            zd = pool_sb.tile([P, 512], BF, tag="g_zd", name="g_zd")
            nc.vector.tensor_tensor(out=zd[:, :qn], in0=z2[:, :qn],
                                    in1=d[:, :qn], op=ALU.mult)
            xs = pool_sb.tile([P, 512], BF, tag="g_xs", name="g_xs")
            nc.vector.tensor_tensor(out=xs[:, :qn], in0=n_[:, :qn],
                                    in1=zd[:, :qn], op=ALU.add)
            nc.vector.tensor_scalar(out=outT_ap[:, sl], in0=xs[:, :qn],
                                    scalar1=0.0, scalar2=None, op0=ALU.max)

    def elu(self, pool_sb, ps_ap, bias_col, out_ap, qn):
        # elu(x) = max(x,0) + exp(min(x,0)) - 1,  x = psum + bias
        nc = self.nc
        el = pool_sb.tile([P, 512], BF, tag="e_el", name="e_el")
        nc.vector.tensor_scalar(out=el[:, :qn], in0=ps_ap, scalar1=bias_col,
                                scalar2=0.0, op0=ALU.add, op1=ALU.min)
        ex = pool_sb.tile([P, 512], BF, tag="e_ex", name="e_ex")
        nc.scalar.activation(ex[:, :qn], el[:, :qn], AF.Exp)
        mx = pool_sb.tile([P, 512], BF, tag="e_mx", name="e_mx")
        nc.vector.tensor_scalar(out=mx[:, :qn], in0=ps_ap, scalar1=bias_col,
                                scalar2=0.0, op0=ALU.add, op1=ALU.max)
        sm = pool_sb.tile([P, 512], BF, tag="e_sm", name="e_sm")
        nc.gpsimd.tensor_tensor(out=sm[:, :qn], in0=mx[:, :qn],
                                in1=ex[:, :qn], op=ALU.add)
        nc.gpsimd.tensor_scalar(out=out_ap, in0=sm[:, :qn], scalar1=-1.0,
                                scalar2=None, op0=ALU.add)

    # ---------------- edge layer (1 or 2) ----------------
    def edge_layer(self, tc, layer):
        nc, cfg = self.nc, self.cfg
        WPM = MGN // P
        esub = cfg.esub_sw
        n_sub_total = cfg.s_e // P
        sub_sw = np.repeat(np.arange(cfg.n_sw), esub)
        sw_first = np.concatenate([[0], np.cumsum(esub)])
        ch_sub = cfg.ch_sub
        sident = self.sW["identity"]
        MMX = nc.tensor.matmul

        # PSUM bank budget (8 banks of 2KB):
        #  L1: wps 2 + gpsA 2 + pinA 1 + hsw 1 + tps 1 + pcol 1 = 8
        #  L2: gpsA 2 + pinA 1 + hsw 2 + tps 1 + pcol 1 = 7
        from contextlib import ExitStack
        with ExitStack() as es:
            pg = es.enter_context(tc.tile_pool(name=f"l{layer}g", bufs=2))
            prgather = es.enter_context(
                tc.tile_pool(name=f"l{layer}rg", bufs=4))
            psub = es.enter_context(tc.tile_pool(name=f"l{layer}s", bufs=3))
            pwide = es.enter_context(tc.tile_pool(name=f"l{layer}w", bufs=2))
            pmg = es.enter_context(tc.tile_pool(name=f"l{layer}mg", bufs=2))
            if layer == 1:
                pwps = es.enter_context(
                    tc.tile_pool(name=f"l{layer}wps", bufs=2, space="PSUM"))
            phsw = es.enter_context(
                tc.tile_pool(name=f"l{layer}hsw", bufs=2, space="PSUM"))
            ppsg = es.enter_context(
                tc.tile_pool(name=f"l{layer}gps", bufs=2, space="PSUM"))
            ppc = es.enter_context(
                tc.tile_pool(name=f"l{layer}pc", bufs=1, space="PSUM"))

            aggT_bufs = {}
            mg_done = {}
            hsw_tiles = {}

            def sw_epilogue(sw, hps):
                mg = (sw * SW) // MGN
                if mg not in aggT_bufs:
                    aggT_bufs[mg] = pmg.tile([P, MGN], BF, tag="aggT",
                                             name="aggT")
                aggT = aggT_bufs[mg]
                off = (sw * SW) % MGN
                sden = psub.tile([P, 1], F32, tag="sden", name="sden")
                nc.vector.tensor_scalar(out=sden[:, :], in0=hps[:, H:H + 1],
                                        scalar1=1e-16, scalar2=None,
                                        op0=ALU.add)
                srec = psub.tile([P, 1], F32, tag="srec", name="srec")
                nc.vector.reciprocal(srec[:, :], sden[:, :])
                if self.dbg and layer == 1:
                    nc.sync.dma_start(out=self.dbg_den[:, sw:sw + 1],
                                      in_=sden[:, :])
                aggN = psub.tile([P, H], BF, tag="aggN", name="aggN")
                nc.scalar.activation(aggN[:, :], hps[:, :H], AF.Copy,
                                     scale=srec[:, :])
                nc.sync.dma_start_transpose(aggT[:, off:off + P], aggN[:, :])
                mg_done[mg] = mg_done.get(mg, 0) + 1
                if mg_done[mg] == MGN // SW:
                    mg_epilogue(mg, aggT_bufs.pop(mg))

            def mg_epilogue(mg, aggT):
                m0 = mg * MGN
                hT = pmg.tile([P, MGN], BF, tag="hT", name="hT")
                for q in range(MGN // 512):
                    if layer == 1:
                        ps = pwps.tile([P, 512], F32, tag="wps", name="wps",
                                       space="PSUM")
                    else:
                        ps = ppsg.tile([P, 512], F32, tag="gpsA", name="wps",
                                       space="PSUM")
                    MMX(ps[:, :], lhsT=(self.sW["W2T"] if layer == 1
                                        else self.sW["convWT"])[:, :],
                        rhs=aggT[:, q * 512:(q + 1) * 512], start=True,
                        stop=True)
                    if layer == 1:
                        self.elu(pmg, ps[:, :], self.cc("gbias"),
                                 hT[:, q * 512:(q + 1) * 512], 512)
                    else:
                        nc.scalar.activation(hT[:, q * 512:(q + 1) * 512],
                                             ps[:, :], AF.Relu,
                                             bias=self.cc("cbias"))
                if self.dbg and layer == 1:
                    nc.sync.dma_start(out=self.dbg_h[:, m0:m0 + MGN],
                                      in_=hT[:, :])
                xprevT = pmg.tile([P, MGN], BF, tag="xprevT", name="xprevT")
                nc.sync.dma_start(
                    out=xprevT[:, :],
                    in_=(self.x1T_d if layer == 1
                         else self.x2T_d)[:, m0:m0 + MGN])
                xnewT = pmg.tile([P, MGN], BF, tag="xnewT", name="xnewT")
                tg = "1" if layer == 1 else "2"
                self.gru(pmg, ppsg, self.sW["Wih" + tg + "T"],
                         self.sW["Whh" + tg + "T"], tg, hT[:, :], xprevT[:, :],
                         xnewT[:, :], MGN)
                aug = pmg.tile([P, WPM * (H + 1)], BF, tag="aug", name="aug")
                pcd = ppc.tile([P, 48], F32, tag="pcol", name="pcd",
                               space="PSUM")
                psc = pcd[:, 0:WPM]
                psd = pcd[:, WPM:2 * WPM]
                for w8 in range(WPM):
                    sl = slice(w8 * P, (w8 + 1) * P)
                    xtb = psub.tile([P, P], BF, tag="xtb", name="xtb")
                    nc.sync.dma_start_transpose(xtb[:, :], xnewT[:, sl])
                    nc.vector.tensor_copy(
                        aug[:, w8 * (H + 1):w8 * (H + 1) + H], xtb[:, :])
                    MMX(psc[:, w8:w8 + 1], lhsT=xnewT[:, sl],
                        rhs=self.ch16("cattsrc") if layer == 1
                        else self.ch16("cattmsrc"), start=True, stop=True,
                        skip_group_check=True)
                    if layer == 1:
                        MMX(psd[:, w8:w8 + 1], lhsT=xnewT[:, sl],
                            rhs=self.ch16("cattdst"), start=True, stop=True,
                            skip_group_check=True)
                augv = aug[:, :].rearrange("p (w f) -> p w f", w=WPM)
                nc.vector.tensor_copy(augv[:, :, H:H + 1],
                                      psc.rearrange("p (w o) -> p w o", o=1))
                if layer == 1:
                    nc.vector.tensor_copy(
                        self.a2sb[:, mg * WPM:(mg + 1) * WPM], psd)
                aug_d = self.x2aug_d if layer == 1 else self.x3aug_d
                dview = aug_d[m0:m0 + MGN, :].rearrange("(w p) f -> p w f",
                                                        p=P)
                nc.sync.dma_start(out=dview, in_=augv)
                if layer == 1:
                    nc.sync.dma_start(out=self.x2T_d[:, m0:m0 + MGN],
                                      in_=xnewT[:, :])

            n_ch = math.ceil(n_sub_total / ch_sub)
            for ch in range(n_ch):
                st0 = ch * ch_sub
                st1 = min(st0 + ch_sub, n_sub_total)
                k = st1 - st0
                if layer == 1:
                    xgc = pg.tile([cfg.in_pad, ch_sub * P], BF, tag="xgc",
                                  name="xgc")
                    nc.sync.dma_start(out=xgc[:, :k * P],
                                      in_=self.xgT[:, st0 * P:st1 * P])
                    eac = pg.tile([cfg.ed, ch_sub * P], BF, tag="eac",
                                  name="eac")
                    nc.sync.dma_start(out=eac[:, :k * P],
                                      in_=self.eaT[:, st0 * P:st1 * P])
                drc = pg.tile([P, ch_sub], F32, tag="drc", name="drc")
                nc.sync.dma_start(out=drc[:, :k], in_=self.dstrel[:, st0:st1])
                if layer == 2:
                    gix = pg.tile([P, ch_sub], I32, tag="gix", name="gix")
                    nc.sync.dma_start(out=gix[:, :k],
                                      in_=self.gidx2[:, st0:st1])
                drcrep = pg.tile([P, ch_sub * P], BF, tag="drcrep",
                                 name="drcrep")
                nc.sync.dma_start(
                    out=drcrep[:, :k * P],
                    in_=self.dstrel_row[:, st0 * P:st1 * P].to_broadcast(
                        [P, k * P]))

                # ---- stage A: wide lin1/gate transforms (layer 1) ----
                if layer == 1:
                    xj1c = pwide.tile([P, ch_sub * P], BF, tag="xj1c",
                                      name="xj1c")
                    xj1em = pwide.tile([P, ch_sub * P], BF, tag="xj1em",
                                       name="xj1em")
                    hec = pwide.tile([P, ch_sub * P], BF, tag="hec",
                                     name="hec")
                    for w4 in range(math.ceil(k * P / 512)):
                        qs = slice(w4 * 512, min((w4 + 1) * 512, k * P))
                        qn = qs.stop - qs.start
                        psx = pwps.tile([P, 512], F32, tag="wps", name="psx",
                                        space="PSUM")
                        MMX(psx[:, :qn], lhsT=self.sW["Wlin1T"][:, :],
                            rhs=xgc[:, qs], start=True, stop=True)
                        nc.scalar.activation(xj1c[:, qs], psx[:, :qn],
                                             AF.Lrelu, bias=self.cc("b1"),
                                             alpha=0.01)
                        psh = pwps.tile([P, 512], F32, tag="wps", name="psh",
                                        space="PSUM")
                        MMX(psh[:, :qn], lhsT=self.sW["W1aT"][:, :],
                            rhs=xj1c[:, qs], start=True, stop=False)
                        MMX(psh[:, :qn], lhsT=self.sW["W1bT"][:, :],
                            rhs=eac[:, qs], start=False, stop=True)
                        nc.scalar.activation(hec[:, qs], psh[:, :qn],
                                             AF.Lrelu, alpha=0.01)
                        # edge-major lin1 (bias via the constant-1 pad row)
                        psm = pwps.tile([P, 512], F32, tag="wps", name="psm",
                                        space="PSUM")
                        for i in range(math.ceil(qn / P)):
                            c0 = qs.start + i * P
                            MMX(psm[:, i * P:(i + 1) * P],
                                lhsT=xgc[:, c0:c0 + P],
                                rhs=self.sW["Wlin1R"][:, :], start=True,
                                stop=True, skip_group_check=True)
                        nc.scalar.activation(xj1em[:, qs], psm[:, :qn],
                                             AF.Lrelu, alpha=0.01)
                else:
                    rgc = prgather.tile([P, ch_sub * (H + 1)], BF, tag="rgc",
                                        name="rgc")
                    for st in range(st0, st1):
                        j = st - st0
                        nc.gpsimd.indirect_dma_start(
                            out=rgc[:, j * (H + 1):(j + 1) * (H + 1)],
                            out_offset=None,
                            in_=self.agf_d[:, :],
                            in_offset=bass.IndirectOffsetOnAxis(
                                ap=gix[:, j:j + 1], axis=0))

                # ---- stage B: per-subtile alpha accumulation ----
                pcd_a = ppc.tile([P, 48], F32, tag="pcol", name="pcda",
                                 space="PSUM")
                pa = pcd_a[:, 0:ch_sub]
                mts = psub.tile([P, ch_sub * P], BF, tag="mts", name="mts")
                for st in range(st0, st1):
                    j = st - st0
                    sw = int(sub_sw[st])
                    mt = mts[:, j * P:(j + 1) * P]
                    nc.vector.tensor_scalar(out=mt, in0=drcrep[:, j * P:(j + 1) * P],
                                            scalar1=self.cc("iop_lo"),
                                            scalar2=None, op0=ALU.is_equal)
                    if layer == 1:
                        MMX(pa[:, j:j + 1], lhsT=hec[:, j * P:(j + 1) * P],
                            rhs=self.ch16("attl"), start=True, stop=False,
                            skip_group_check=True)
                        MMX(pa[:, j:j + 1], lhsT=mt,
                            rhs=self.r1sb[:, sw:sw + 1], start=False,
                            stop=True, skip_group_check=True)
                    else:
                        MMX(pa[:, j:j + 1], lhsT=mt,
                            rhs=self.a2sb[:, sw:sw + 1], start=True,
                            stop=False, skip_group_check=True)
                        MMX(pa[:, j:j + 1], lhsT=sident[:, :],
                            rhs=rgc[:, j * (H + 1) + H:(j + 1) * (H + 1)],
                            start=False, stop=True, skip_group_check=True)

                # ---- stage C: batched lrelu+exp over chunk alphas ----
                arel = psub.tile([P, ch_sub], BF, tag="arel", name="arel")
                nc.scalar.activation(arel[:, :k], pa[:, :k], AF.Lrelu,
                                     alpha=0.01)
                aex = psub.tile([P, ch_sub], F32, tag="aex", name="aex")
                nc.scalar.activation(aex[:, :k], arel[:, :k], AF.Exp)
                aex16 = psub.tile([P, ch_sub], BF, tag="aex16", name="aex16")
                nc.gpsimd.tensor_copy(aex16[:, :k], aex[:, :k])

                # ---- stage D: scale messages + scatter ----
                for st in range(st0, st1):
                    j = st - st0
                    sw = int(sub_sw[st])
                    first = st == sw_first[sw]
                    last = st == sw_first[sw + 1] - 1
                    if first:
                        hsw_tiles[sw] = phsw.tile([P, H + 1], F32, tag="hsw",
                                                  name="hsw", space="PSUM")
                    hps = hsw_tiles[sw]
                    m = psub.tile([P, P], BF, tag="m", name="m")
                    nc.gpsimd.tensor_scalar(out=m[:, :],
                                            in0=self.sW["iotaRowLo"][:, :],
                                            scalar1=drc[:, j:j + 1],
                                            scalar2=None, op0=ALU.is_equal)
                    msg = psub.tile([P, H + 1], BF, tag="msg", name="msg")
                    if layer == 1:
                        nc.vector.tensor_scalar(out=msg[:, :H],
                                                in0=xj1em[:, j * P:(j + 1) * P],
                                                scalar1=aex[:, j:j + 1],
                                                scalar2=None, op0=ALU.mult)
                    else:
                        nc.vector.tensor_scalar(
                            out=msg[:, :H],
                            in0=rgc[:, j * (H + 1):j * (H + 1) + H],
                            scalar1=aex[:, j:j + 1], scalar2=None,
                            op0=ALU.mult)
                    nc.gpsimd.tensor_copy(msg[:, H:H + 1], aex16[:, j:j + 1])
                    MMX(hps[:, :], lhsT=m[:, :], rhs=msg[:, :], start=first,
                        stop=last, skip_group_check=True)
                    if last:
                        sw_epilogue(sw, hsw_tiles.pop(sw))

    def allgather(self):
        nc, cfg = self.nc, self.cfg
        rch = cfg.n_pad // cfg.nch
        for q in range(cfg.nch):
            nc.gpsimd.collective_compute(
                "AllGather", ALU.bypass,
                replica_groups=[list(range(NCORES))],
                ins=[self.x2aug_d[q * rch:(q + 1) * rch, :].opt()],
                outs=[self.agf_d[q * NCORES * rch:(q + 1) * NCORES * rch,
                                 :].opt()])

    # ---------------- readout ----------------
    def readout(self, tc):
        nc, cfg = self.nc, self.cfg
        n_sub_total = cfg.n_pad // P
        sub_per_blk = cfg.s_blk // P
        ch_sub = cfg.ch_sub
        sident = self.sW["identity"]
        MMX = nc.tensor.matmul

        # PSUM banks: rops 1 + roaps 1 + gpsA 2 + rohm(2 tags) 2 = 6
        with tc.tile_pool(name="ro", bufs=2) as pro, \
             tc.tile_pool(name="roS", bufs=3) as prs, \
             tc.tile_pool(name="roT", bufs=1) as proT, \
             tc.tile_pool(name="rog", bufs=3) as prg, \
             tc.tile_pool(name="rops", bufs=1, space="PSUM") as prps, \
             tc.tile_pool(name="roaps", bufs=1, space="PSUM") as paps, \
             tc.tile_pool(name="rogps", bufs=2, space="PSUM") as ppsg, \
             tc.tile_pool(name="rohm", bufs=1, space="PSUM") as phm:

            outT = proT.tile([P, cfg.b_pad], BF, tag="outT", name="outT")
            helT = proT.tile([P, cfg.b_pad], BF, tag="helT", name="helT")
            admol = [proT.tile([P, cfg.mw_n], BF, tag=f"admol{t}",
                               name=f"admol{t}")
                     for t in range(NUM_TIMESTEPS)]

            def mol_sw_epilogue(tstep, msw, tiles):
                for half, hps in enumerate(tiles):
                    mw = 2 * msw + half
                    off = mw * P
                    if tstep < 0:
                        agg = prg.tile([P, H], BF, tag="magg", name="magg")
                        nc.vector.tensor_copy(agg[:, :], hps[:, :H])
                        aggt = prg.tile([P, P], BF, tag="maggt", name="maggt")
                        nc.sync.dma_start_transpose(aggt[:, :], agg[:, :])
                        nc.scalar.activation(outT[:, off:off + P], aggt[:, :],
                                             AF.Relu)
                        continue
                    sden = prg.tile([P, 1], F32, tag="msden", name="msden")
                    nc.vector.tensor_scalar(out=sden[:, :],
                                            in0=hps[:, H:H + 1],
                                            scalar1=1e-16, scalar2=None,
                                            op0=ALU.add)
                    srec = prg.tile([P, 1], F32, tag="msrec", name="msrec")
                    nc.vector.reciprocal(srec[:, :], sden[:, :])
                    aggN = prg.tile([P, H], BF, tag="maggN", name="maggN")
                    nc.vector.tensor_scalar(out=aggN[:, :], in0=hps[:, :H],
                                            scalar1=srec[:, :], scalar2=None,
                                            op0=ALU.mult)
                    nc.sync.dma_start_transpose(helT[:, off:off + P],
                                                aggN[:, :])

            def mol_pass(tstep):
                hm_tiles = {}
                for ch in range(math.ceil(n_sub_total / ch_sub)):
                    st0 = ch * ch_sub
                    st1 = min(st0 + ch_sub, n_sub_total)
                    k = st1 - st0
                    rg = pro.tile([P, ch_sub * (H + 1)], BF, tag="rg",
                                  name="rrg")
                    nc.sync.dma_start(
                        out=rg[:, :k * (H + 1)].rearrange(
                            "p (j f) -> p j f", j=k),
                        in_=self.x3aug_d[st0 * P:st1 * P, :].rearrange(
                            "(j p) f -> p j f", p=P))
                    mrl = pro.tile([P, ch_sub], F32, tag="mrl", name="mrl")
                    nc.sync.dma_start(out=mrl[:, :k],
                                      in_=self.molrel[:, st0:st1])
                    if tstep >= 0:
                        mrlrep = pro.tile([P, ch_sub * P], BF, tag="mrlrep",
                                          name="mrlrep")
                        nc.sync.dma_start(
                            out=mrlrep[:, :k * P],
                            in_=self.molrel_row[:, st0 * P:st1 * P]
                            .to_broadcast([P, k * P]))
                        pa = paps.tile([P, ch_sub], F32, tag="mpa", name="mpa",
                                       space="PSUM")
                        for st in range(st0, st1):
                            j = st - st0
                            msw = st // sub_per_blk
                            mtlo = prs.tile([P, P], BF, tag="mmtlo",
                                            name="mmtlo")
                            nc.vector.tensor_scalar(
                                out=mtlo[:, :],
                                in0=mrlrep[:, j * P:(j + 1) * P],
                                scalar1=self.cc("iop_lo"), scalar2=None,
                                op0=ALU.is_equal)
                            mthi = prs.tile([P, P], BF, tag="mmthi",
                                            name="mmthi")
                            nc.vector.tensor_scalar(
                                out=mthi[:, :],
                                in0=mrlrep[:, j * P:(j + 1) * P],
                                scalar1=self.cc("iop_hi"), scalar2=None,
                                op0=ALU.is_equal)
                            MMX(pa[:, j:j + 1], lhsT=mtlo,
                                rhs=admol[tstep][:, 2 * msw:2 * msw + 1],
                                start=True, stop=False, skip_group_check=True)
                            MMX(pa[:, j:j + 1], lhsT=mthi,
                                rhs=admol[tstep][:, 2 * msw + 1:2 * msw + 2],
                                start=False, stop=False, skip_group_check=True)
                            MMX(pa[:, j:j + 1], lhsT=sident[:, :],
                                rhs=rg[:, j * (H + 1) + H:(j + 1) * (H + 1)],
                                start=False, stop=True, skip_group_check=True)
                        arel = prs.tile([P, ch_sub], BF, tag="marel",
                                        name="marel")
                        nc.scalar.activation(arel[:, :k], pa[:, :k], AF.Lrelu,
                                             alpha=0.01)
                        aex = prs.tile([P, ch_sub], F32, tag="maex",
                                       name="maex")
                        nc.scalar.activation(aex[:, :k], arel[:, :k], AF.Exp)
                        aex16 = prs.tile([P, ch_sub], BF, tag="maex16",
                                         name="maex16")
                        nc.gpsimd.tensor_copy(aex16[:, :k], aex[:, :k])
                    for st in range(st0, st1):
                        j = st - st0
                        msw = st // sub_per_blk
                        first = st % sub_per_blk == 0
                        last = (st + 1) % sub_per_blk == 0
                        if first:
                            hm_tiles[msw] = (
                                phm.tile([P, H + 1], F32, tag="hmlo",
                                         name="hmlo", space="PSUM"),
                                phm.tile([P, H + 1], F32, tag="hmhi",
                                         name="hmhi", space="PSUM"))
                        hlo, hhi = hm_tiles[msw]
                        if tstep >= 0:
                            msg = prs.tile([P, H + 1], BF, tag="mmsg",
                                           name="mmsg")
                            nc.vector.tensor_scalar(
                                out=msg[:, :H],
                                in0=rg[:, j * (H + 1):j * (H + 1) + H],
                                scalar1=aex[:, j:j + 1], scalar2=None,
                                op0=ALU.mult)
                            nc.gpsimd.tensor_copy(msg[:, H:H + 1],
                                                  aex16[:, j:j + 1])
                            rhs_ap = msg[:, :]
                        else:
                            rhs_ap = rg[:, j * (H + 1):j * (H + 1) + H]
                        mlo = prs.tile([P, P], BF, tag="mmlo", name="mmlo")
                        nc.gpsimd.tensor_scalar(out=mlo[:, :],
                                                in0=self.sW["iotaRowLo"][:, :],
                                                scalar1=mrl[:, j:j + 1],
                                                scalar2=None,
                                                op0=ALU.is_equal)
                        mhi = prs.tile([P, P], BF, tag="mmhi", name="mmhi")
                        nc.gpsimd.tensor_scalar(out=mhi[:, :],
                                                in0=self.sW["iotaRowHi"][:, :],
                                                scalar1=mrl[:, j:j + 1],
                                                scalar2=None,
                                                op0=ALU.is_equal)
                        if tstep >= 0:
                            MMX(hlo[:, :], lhsT=mlo[:, :], rhs=rhs_ap,
                                start=first, stop=last, skip_group_check=True)
                            MMX(hhi[:, :], lhsT=mhi[:, :], rhs=rhs_ap,
                                start=first, stop=last, skip_group_check=True)
                        else:
                            MMX(hlo[:, :H], lhsT=mlo[:, :], rhs=rhs_ap,
                                start=first, stop=last, skip_group_check=True)
                            MMX(hhi[:, :H], lhsT=mhi[:, :], rhs=rhs_ap,
                                start=first, stop=last, skip_group_check=True)
                        if last:
                            mol_sw_epilogue(tstep, msw, hm_tiles.pop(msw))
                if tstep >= 0:
                    # batched molW transform + elu + GRU over all molecules
                    hel = prg.tile([P, cfg.b_pad], BF, tag="mhel",
                                   name="mhel")
                    for qb in range(math.ceil(cfg.b_pad / 512)):
                        qs = slice(qb * 512, min((qb + 1) * 512, cfg.b_pad))
                        qn = qs.stop - qs.start
                        psh = prps.tile([P, 512], F32, tag="mwps",
                                        name="mwps", space="PSUM")
                        MMX(psh[:, :qn], lhsT=self.sW["molWT"][:, :],
                            rhs=helT[:, qs], start=True, stop=True)
                        self.elu(prg, psh[:, :qn], self.cc("molbias"),
                                 hel[:, qs], qn)
                    self.gru(prg, ppsg, self.sW["WihmT"], self.sW["WhhmT"],
                             "m", hel[:, :], outT[:, :], outT[:, :],
                             cfg.b_pad)

            def write_admol(tstep):
                pad = prps.tile([P, cfg.mw_n], F32, tag="mwps", name="amps",
                                space="PSUM")
                for mw in range(cfg.mw_n):
                    MMX(pad[:, mw:mw + 1], lhsT=outT[:, mw * P:(mw + 1) * P],
                        rhs=self.ch16("cattmdst"), start=True, stop=True,
                        skip_group_check=True)
                nc.vector.tensor_copy(admol[tstep][:, :], pad[:, :])

            import os
            ro_lvl = int(os.environ.get("K_RO", "3"))
            mol_pass(-1)
            if ro_lvl >= 2:
                for t in range(min(NUM_TIMESTEPS, ro_lvl - 1)):
                    write_admol(t)
                    mol_pass(t)

            predsb = proT.tile([1, cfg.b_pad], F32, tag="pred", name="pred")
            for q in range(math.ceil(cfg.b_pad / 512)):
                qs = slice(q * 512, min((q + 1) * 512, cfg.b_pad))
                qn = qs.stop - qs.start
                ps = prps.tile([P, 512], F32, tag="mwps", name="finps",
                               space="PSUM")
                MMX(ps[:, :qn], lhsT=self.sW["Wlin2T"][:, :], rhs=outT[:, qs],
                    start=True, stop=True)
                o2 = pro.tile([P, 512], BF, tag="o2", name="o2")
                nc.vector.tensor_scalar(out=o2[:, :qn], in0=ps[:, :qn],
                                        scalar1=self.cc("b2"), scalar2=None,
                                        op0=ALU.add)
                ph = prps.tile([P, 512], F32, tag="mwps", name="finph",
                               space="PSUM")
                MMX(ph[0:1, :qn], lhsT=self.sW["WheadT"][:, :], rhs=o2[:, :qn],
                    start=True, stop=True)
                nc.vector.tensor_scalar(out=predsb[:, qs], in0=ph[0:1, :qn],
                                        scalar1=float(self.b_head),
                                        scalar2=None, op0=ALU.add)
            nc.sync.dma_start(out=self.outp[:, :], in_=predsb[:, :])

    def build(self, phases=5):
        from contextlib import ExitStack
        self.declare()
        with tile.TileContext(self.nc) as tc:
            with ExitStack() as stack:
                self.load_weights(tc, stack)
                self.phase0(tc)
                if phases >= 2:
                    self.edge_layer(tc, 1)
                if phases >= 3:
                    self.allgather()
                if phases >= 4:
                    self.edge_layer(tc, 2)
                if self.dbg:
                    with tc.tile_pool(name="dbgp", bufs=1) as pdbg:
                        nc = self.nc
                        r1c = pdbg.tile([P, self.cfg.w_n], F32, tag="r1c",
                                        name="r1c")
                        nc.vector.tensor_copy(r1c[:, :], self.r1sb[:, :])
                        nc.sync.dma_start(out=self.dbg_r1[:, :], in_=r1c[:, :])
                        a2c = pdbg.tile([P, self.cfg.w_n], F32, tag="a2c",
                                        name="a2c")
                        nc.vector.tensor_copy(a2c[:, :], self.a2sb[:, :])
                        nc.sync.dma_start(out=self.dbg_a2[:, :], in_=a2c[:, :])
                        nc.sync.dma_start(out=self.dbg_x1[:, :],
                                          in_=self.x1T_d[:, :])
                        nc.sync.dma_start(out=self.dbg_x2aug[:, :],
                                          in_=self.x2aug_d[:, :])
                        nc.sync.dma_start(out=self.dbg_x3aug[:, :],
                                          in_=self.x3aug_d[:, :])
                if phases >= 5:
                    self.readout(tc)
                else:
                    with tc.tile_pool(name="stub", bufs=1) as pstub:
                        z = pstub.tile([1, self.cfg.b_pad], F32, tag="z",
                                       name="z")
                        self.nc.gpsimd.memset(z[:, :], 0.0)
                        self.nc.sync.dma_start(out=self.outp[:, :],
                                               in_=z[:, :])
        self.nc.finalize()
        return self.nc


def build(cfg, cvec_idx, b_head, c16_idx=None):
    import os
    return Builder(cfg, cvec_idx, c16_idx, b_head).build(
        phases=int(os.environ.get("K_PHASES", "5")))


def kernel(**inputs):
    cfg, in_maps, meta = preprocess(inputs, NCORES)
    nc = build(cfg, meta["cvec_idx"], meta["b_head"], meta["c16_idx"])
    res = run_bass_kernel_spmd(nc, in_maps, core_ids=list(range(NCORES)))
    outs = []
    for c in range(NCORES):
        outs.append(res.results[c]["out"].reshape(-1)[:cfg.b_core])
    return np.concatenate(outs).astype(np.float32)
